# revision 1
# baseline (speedup 1.0000x reference)
"""CQAttention (trilinear context-query attention) Bass kernel for TRN2.

Full-input contract: kernel(**inputs) takes the unsharded tensors
  C (1024, 64, 256), Q (512, 64, 256), w4C (256,1), w4Q (256,1),
  w4mlu (1,1,256), bias (1,)
and returns out (64, 1024, 1024) fp32, matching the reference

  C,Q -> batch-major; S = C@w4C + (Q@w4Q)^T + (C*w4mlu)@Q^T + bias
  S1 = softmax_q(S); S2 = softmax_c(S)
  A = S1@Q ; B = (S1@S2^T)@C
  out = concat([C, A, C*A, C*B], -1) transposed to (B, 4D, Lc)

Sharding: data-parallel over batch, 8 batch items per NeuronCore.

Algebra used on-chip (per batch item):
  * bias cancels in both softmaxes (constant shift) -> dropped.
  * e0 = exp(C@w4C), e1 = exp(Q@w4Q), E0 = exp((C*w4mlu)@Q^T) so that
    exp(S) = e0[c] * E0[c,q] * e1[q].
  * S1 = diag(1/rs) E0 diag(e1),  rs  = E0 @ e1          (e0 cancels)
  * S2 = diag(e0) E0 diag(1/cs),  cs  = E0^T @ e0        (e1 cancels)
  * A    = diag(1/rs) (E0 @ (diag(e1) Q))
  * S2^T C = diag(1/cs) (E0^T @ (diag(e0) C))
  * B    = S1 @ (S2^T C) = diag(1/rs) (E0 @ (diag(e1/cs) (E0^T (diag(e0) C))))
  * (S1@S2^T)@C reassociated as S1@(S2^T@C): halves the matmul FLOPs.
  Everything is computed transposed ([feature, context] layout) so output
  DMA rows are contiguous in DRAM.
"""

import numpy as np

LC, LQ, B, D = 1024, 512, 64, 256
NCORES = 8
BPC = B // NCORES  # batch items per core
P = 128
MC = LC // P  # 8 context chunks
TQ = LQ // P  # 4 query chunks
KD = D // P   # 2 feature chunks

# float32r: single-pass relaxed-precision fp32 matmul (1 cyc/row at N>=256)
# float32:  exact two-pass fp32 matmul (4 cyc/row)
MM_RELAXED = True

_CACHE = {}


def _ensure_path():
    import sys
    for p in ("/opt/trn_rl_repo",):
        if p not in sys.path:
            sys.path.insert(0, p)


def _build_nc(mm_relaxed=MM_RELAXED):
    _ensure_path()
    import concourse.bass as bass
    import concourse.bacc as bacc
    import concourse.mybir as mybir
    from concourse import tile, masks

    f32 = mybir.dt.float32
    mmdt = mybir.dt.float32r if mm_relaxed else f32
    Exp = mybir.ActivationFunctionType.Exp
    Copy = mybir.ActivationFunctionType.Copy
    mult = mybir.AluOpType.mult
    AxX = mybir.AxisListType.X
    add = mybir.AluOpType.add

    def r(ap):
        return ap.bitcast(mmdt)

    nc = bacc.Bacc()
    C_d = nc.dram_tensor("C", [LC, BPC, D], f32, kind="ExternalInput")
    Q_d = nc.dram_tensor("Q", [LQ, BPC, D], f32, kind="ExternalInput")
    w4C_d = nc.dram_tensor("w4C", [D, 1], f32, kind="ExternalInput")
    w4Q_d = nc.dram_tensor("w4Q", [D, 1], f32, kind="ExternalInput")
    w4mlu_d = nc.dram_tensor("w4mlu", [1, 1, D], f32, kind="ExternalInput")
    out_d = nc.dram_tensor("out", [BPC, 4 * D, LC], f32, kind="ExternalOutput")

    with tile.TileContext(nc) as tc:
        import contextlib

        with contextlib.ExitStack() as ctx:
            ep = ctx.enter_context

            consts = ep(tc.tile_pool(name="consts", bufs=1))
            import os as _os0
            cn_pool = ep(tc.tile_pool(name="cn", bufs=int(_os0.environ.get("K_CN","2"))))
            qn_pool = ep(tc.tile_pool(name="qn", bufs=int(_os0.environ.get("K_CN","2"))))
            ct_pool = ep(tc.tile_pool(name="ct", bufs=2))
            ctr_pool = ep(tc.tile_pool(name="ctr", bufs=1))
            qt_pool = ep(tc.tile_pool(name="qt", bufs=int(_os0.environ.get("K_QT","1"))))
            qmt_pool = ep(tc.tile_pool(name="qmt", bufs=int(_os0.environ.get("K_QT","1"))))
            ce_pool = ep(tc.tile_pool(name="ce", bufs=2))
            qe_pool = ep(tc.tile_pool(name="qe", bufs=2))
            e0_pool = ep(tc.tile_pool(name="e0p", bufs=2))
            e0t_pool = ep(tc.tile_pool(name="e0tp", bufs=2))
            h2_pool = ep(tc.tile_pool(name="h2", bufs=2))
            rsbr_pool = ep(tc.tile_pool(name="rsbr", bufs=1))
            at_pool = ep(tc.tile_pool(name="at", bufs=int(_os0.environ.get("K_AT","2"))))
            bt_pool = ep(tc.tile_pool(name="bt", bufs=int(_os0.environ.get("K_AT","2"))))
            # O2 reuses ce_pool slots (Ce dead after P2); O3 reuses e0t slots
            o2_pool = ce_pool
            o3_pool = e0t_pool
            small_pool = ep(tc.tile_pool(name="small", bufs=4))
            scr_pool = ep(tc.tile_pool(name="scr", bufs=1))
            row_pool = ep(tc.tile_pool(name="rows", bufs=1))

            import os as _os2
            _psa = int(_os2.environ.get("K_PSA", "4"))
            _psrow = int(_os2.environ.get("K_PSROW", "2"))
            psA = ep(tc.tile_pool(name="psA", bufs=_psa, space="PSUM"))
            psB = ep(tc.tile_pool(name="psB", bufs=int(_os2.environ.get("K_PSB","2")), space="PSUM"))
            psRow = ep(tc.tile_pool(name="psRow", bufs=_psrow, space="PSUM"))

            # ---- per-core constants ----
            ident = consts.tile([P, P], f32)
            masks.make_identity(nc, ident[:])
            ones_row = consts.tile([1, P], f32)
            nc.vector.memset(ones_row[:], 1.0)
            ones_r = consts.tile([1, P], f32)
            nc.scalar.copy(r(ones_r[:]), ones_row[:])
            w4mlu_pp = consts.tile([P, KD], f32)
            nc.sync.dma_start(
                w4mlu_pp[:], w4mlu_d[0, 0, :].rearrange("(k p) -> p k", p=P)
            )
            # matvec weights replicated across partitions via broadcast DMA
            w4Cb = consts.tile([P, D], f32)
            nc.sync.dma_start(
                w4Cb[:],
                w4C_d[:, 0].rearrange("(a d) -> a d", a=1).broadcast_to([P, D]),
            )
            w4Qb = consts.tile([P, D], f32)
            nc.sync.dma_start(
                w4Qb[:],
                w4Q_d[:, 0].rearrange("(a d) -> a d", a=1).broadcast_to([P, D]),
            )

            import os as _os
            _nb = int(_os.environ.get("K_EMIT_BATCHES", str(BPC)))
            _ph = int(_os.environ.get("K_EMIT_PHASE", "99"))
            class _ActShim:
                def tensor_copy(self, out, in_):
                    return nc.scalar.copy(out, in_)
                def tensor_scalar_mul(self, out, in_, s):
                    return nc.scalar.activation(out, in_, Copy, scale=s)
            _act_shim = _ActShim()
            _ect = nc.vector if _os.environ.get("K_ECT", "act") == "dve" else _act_shim
            _eh2 = nc.vector if _os.environ.get("K_EH2", "act") == "dve" else _act_shim
            _pro_state = {}

            def _prologue(b):
                # loads (natural layouts) + gpsimd matvec mults for batch b
                Cn = cn_pool.tile([P, MC * D], f32, tag="cn")
                for m in range(MC):
                    nc.sync.dma_start(
                        Cn[:, m * D:(m + 1) * D], C_d[m * P:(m + 1) * P, b, :]
                    )
                Qn = qn_pool.tile([P, TQ * D], f32, tag="qn")
                for t in range(TQ):
                    nc.sync.dma_start(
                        Qn[:, t * D:(t + 1) * D], Q_d[t * P:(t + 1) * P, b, :]
                    )
                scr = scr_pool.tile([P, MC * D], f32, tag="scrA")
                w4Cb_bc = w4Cb[:].rearrange("p (a d) -> p a d", a=1) \
                    .broadcast_to([P, MC // 2, D])
                for h in range(2):
                    hs = slice(h * (MC // 2) * D, (h + 1) * (MC // 2) * D)
                    nc.gpsimd.tensor_tensor(
                        scr[:, hs].rearrange("p (m d) -> p m d", m=MC // 2),
                        Cn[:, hs].rearrange("p (m d) -> p m d", m=MC // 2),
                        w4Cb_bc, mult,
                    )
                scr1 = scr_pool.tile([P, MC * D], f32, tag="scrA")
                w4Qb_bc = w4Qb[:].rearrange("p (a d) -> p a d", a=1) \
                    .broadcast_to([P, TQ, D])
                nc.gpsimd.tensor_tensor(
                    scr1[:, 0:TQ * D].rearrange("p (t d) -> p t d", t=TQ),
                    Qn[:].rearrange("p (t d) -> p t d", t=TQ),
                    w4Qb_bc, mult,
                )
                _pro_state[b] = (Cn, Qn, scr, scr1)

            _prologue(0)
            for b in range(_nb):
                Cn, Qn, scr, scr1 = _pro_state.pop(b)
                if b + 1 < _nb:
                    _prologue(b + 1)
                # ---- transposes: CT [d,(k,c)], QT [d,(k,q)] ----
                CT = ct_pool.tile([P, KD * LC], f32)
                CTr = ctr_pool.tile([P, KD * LC], f32)
                for k in range(KD):
                    for mg in range(0, MC, 4):
                        pst = psA.tile([P, 4 * P], f32, tag="psA")
                        for j in range(4):
                            m = mg + j
                            nc.tensor.transpose(
                                pst[:, j * P:(j + 1) * P],
                                Cn[:, m * D + k * P: m * D + (k + 1) * P],
                                ident[:],
                            )
                        _ect.tensor_copy(
                            CT[:, k * LC + mg * P: k * LC + (mg + 4) * P], pst[:]
                        )
                        nc.vector.tensor_copy(
                            r(CTr[:, k * LC + mg * P: k * LC + (mg + 4) * P]),
                            pst[:],
                        )
                QT = qt_pool.tile([P, KD * LQ], f32)
                for k in range(KD):
                    pst = psA.tile([P, 4 * P], f32, tag="psA")
                    for t in range(TQ):
                        nc.tensor.transpose(
                            pst[:, t * P:(t + 1) * P],
                            Qn[:, t * D + k * P: t * D + (k + 1) * P],
                            ident[:],
                        )
                    _ect.tensor_copy(QT[:, k * LQ: k * LQ + 4 * P], pst[:])

                # matvec reduces + exps (after evacs to keep ACT/DVE queues clear)
                sub0 = small_pool.tile([P, MC], f32)
                for h in range(2):
                    hs = slice(h * (MC // 2) * D, (h + 1) * (MC // 2) * D)
                    nc.vector.tensor_reduce(
                        sub0[:, h * (MC // 2):(h + 1) * (MC // 2)],
                        scr[:, hs].rearrange("p (m d) -> p m d", m=MC // 2),
                        axis=AxX, op=add,
                    )
                e0 = small_pool.tile([P, MC], f32)
                nc.scalar.activation(r(e0[:]), sub0[:], Exp)
                sub1 = small_pool.tile([P, TQ], f32)
                nc.vector.tensor_reduce(
                    sub1[:], scr1[:, 0:TQ * D].rearrange("p (t d) -> p t d", t=TQ),
                    axis=AxX, op=add,
                )
                e1 = small_pool.tile([P, TQ], f32)
                nc.scalar.activation(r(e1[:]), sub1[:], Exp)
                for k in range(KD):
                    nc.sync.dma_start(
                        out_d[b, k * P:(k + 1) * P, :], CT[:, k * LC:(k + 1) * LC]
                    )
                # QmT = QT * w4mlu (per-partition over d)
                QmT = qmt_pool.tile([P, KD * LQ], f32)
                for k in range(KD):
                    nc.vector.tensor_scalar_mul(
                        r(QmT[:, k * LQ:(k + 1) * LQ]),
                        QT[:, k * LQ:(k + 1) * LQ],
                        w4mlu_pp[:, k:k + 1],
                    )

                # Ce = C * e0, Qe = Q * e1 (per-partition scales)
                DA = D + 2
                Ce = ce_pool.tile([P, MC * DA], f32, tag="ceA")
                for m in range(MC):
                    nc.vector.tensor_scalar_mul(
                        r(Ce[:, m * DA:m * DA + D]), Cn[:, m * D:(m + 1) * D],
                        e0[:, m:m + 1],
                    )
                    nc.vector.tensor_copy(
                        r(Ce[:, m * DA + D:m * DA + DA]),
                        e0[:, m:m + 1].broadcast_to([P, 2]),
                    )
                Qe = qe_pool.tile([P, TQ * D], f32)
                for t in range(TQ):
                    nc.vector.tensor_scalar_mul(
                        r(Qe[:, t * D:(t + 1) * D]), Qn[:, t * D:(t + 1) * D],
                        e1[:, t:t + 1],
                    )

                # ---- E0 = exp((C*w)@Q^T) [c,(m,q)] ----
                E0 = e0_pool.tile([P, MC * LQ], f32)
                for m in range(MC):
                    ps = psA.tile([P, LQ], f32, tag="psA")
                    for k in range(KD):
                        nc.tensor.matmul(
                            ps[:],
                            r(CTr[:, k * LC + m * P: k * LC + (m + 1) * P]),
                            r(QmT[:, k * LQ:(k + 1) * LQ]),
                            start=(k == 0),
                            stop=(k == KD - 1),
                        )
                    nc.scalar.activation(r(E0[:, m * LQ:(m + 1) * LQ]), ps[:], Exp)

                # ---- E0T = exp(transposed scores) [q,(t,c)] ----
                E0T = e0t_pool.tile([P, TQ * LC], f32, tag="e0tA")
                for t in range(TQ):
                    for n in range(2):
                        ps = psA.tile([P, 512], f32, tag="psA")
                        for k in range(KD):
                            nc.tensor.matmul(
                                ps[:],
                                r(QmT[:, k * LQ + t * P: k * LQ + (t + 1) * P]),
                                r(CTr[:, k * LC + n * 512: k * LC + (n + 1) * 512]),
                                start=(k == 0),
                                stop=(k == KD - 1),
                            )
                        nc.scalar.activation(
                            r(E0T[:, t * LC + n * 512: t * LC + (n + 1) * 512]),
                            ps[:], Exp,
                        )

                # ---- rs = E0 @ e1 as a row; rsbr = 1/rs replicated ----
                rs_row = row_pool.tile([1, LC], f32, tag="rowA")
                for n in range(2):
                    psr = psRow.tile([1, 512], f32)
                    for t in range(TQ):
                        nc.tensor.matmul(
                            psr[:],
                            r(e1[:, t:t + 1]),
                            r(E0T[:, t * LC + n * 512: t * LC + (n + 1) * 512]),
                            start=(t == 0),
                            stop=(t == TQ - 1),
                        )
                    nc.scalar.copy(r(rs_row[:, n * 512:(n + 1) * 512]), psr[:])
                rsbr = rsbr_pool.tile([P, LC], f32, tag="rsbr")
                for n in range(2):
                    ps = psB.tile([P, 512], f32, tag="psB")
                    nc.tensor.matmul(
                        ps[:], r(ones_r[:]), r(rs_row[:, n * 512:(n + 1) * 512])
                    )
                    nc.vector.reciprocal(rsbr[:, n * 512:(n + 1) * 512], ps[:])

                rec_cse = small_pool.tile([P, TQ], f32)
                dq = small_pool.tile([P, TQ], f32)

                # ---- P2 = E0^T @ Ce ; H2 = dq * P2  [q,(t,d)] ----
                H2 = h2_pool.tile([P, TQ * D], f32)
                for qm in range(TQ):
                    ps = psB.tile([P, 512], f32, tag="psB")
                    for m in range(MC):
                        nc.tensor.matmul(
                            ps[:, 0:DA],
                            r(E0[:, m * LQ + qm * P: m * LQ + (qm + 1) * P]),
                            r(Ce[:, m * DA:(m + 1) * DA]),
                            start=(m == 0),
                            stop=(m == MC - 1),
                        )
                    nc.vector.reciprocal(rec_cse[:, qm:qm + 1], ps[:, D:D + 1])
                    nc.vector.tensor_tensor(
                        dq[:, qm:qm + 1], rec_cse[:, qm:qm + 1], e1[:, qm:qm + 1],
                        mult,
                    )
                    _eh2.tensor_scalar_mul(
                        r(H2[:, qm * D:(qm + 1) * D]), ps[:, 0:D],
                        dq[:, qm:qm + 1],
                    )

                # ---- P1T = Qe^T @ E0T -> AT ; O2 = CT*AT ----
                AT = at_pool.tile([P, KD * LC], f32)
                O2 = o2_pool.tile([P, KD * LC], f32, tag="ceA")
                for m2 in range(KD):
                    for n in range(2):
                        ps = psB.tile([P, 512], f32, tag="psB")
                        for t in range(TQ):
                            nc.tensor.matmul(
                                ps[:],
                                r(Qe[:, t * D + m2 * P: t * D + (m2 + 1) * P]),
                                r(E0T[:, t * LC + n * 512: t * LC + (n + 1) * 512]),
                                start=(t == 0),
                                stop=(t == TQ - 1),
                            )
                        sl = slice(m2 * LC + n * 512, m2 * LC + (n + 1) * 512)
                        nsl = slice(n * 512, (n + 1) * 512)
                        nc.vector.tensor_tensor(AT[:, sl], ps[:], rsbr[:, nsl], mult)
                        nc.gpsimd.tensor_tensor(O2[:, sl], CT[:, sl], AT[:, sl], mult)
                        if n == 1:
                            ksl = slice(m2 * LC, (m2 + 1) * LC)
                            nc.sync.dma_start(
                                out_d[b, 2 * P + m2 * P: 2 * P + (m2 + 1) * P, :],
                                AT[:, ksl],
                            )
                            nc.sync.dma_start(
                                out_d[b, 4 * P + m2 * P: 4 * P + (m2 + 1) * P, :],
                                O2[:, ksl],
                            )

                # ---- P3T = H2^T(as lhsT) @ E0T -> BT ; O3 = CT*BT ----
                BT = bt_pool.tile([P, KD * LC], f32)
                O3 = o3_pool.tile([P, KD * LC], f32, tag="e0tA")
                for m2 in range(KD):
                    for n in range(2):
                        ps = psB.tile([P, 512], f32, tag="psB")
                        for t in range(TQ):
                            nc.tensor.matmul(
                                ps[:],
                                r(H2[:, t * D + m2 * P: t * D + (m2 + 1) * P]),
                                r(E0T[:, t * LC + n * 512: t * LC + (n + 1) * 512]),
                                start=(t == 0),
                                stop=(t == TQ - 1),
                            )
                        sl = slice(m2 * LC + n * 512, m2 * LC + (n + 1) * 512)
                        nsl = slice(n * 512, (n + 1) * 512)
                        nc.vector.tensor_tensor(BT[:, sl], ps[:], rsbr[:, nsl], mult)
                        nc.gpsimd.tensor_tensor(O3[:, sl], CT[:, sl], BT[:, sl], mult)
                        if n == 1:
                            ksl = slice(m2 * LC, (m2 + 1) * LC)
                            nc.sync.dma_start(
                                out_d[b, 6 * P + m2 * P: 6 * P + (m2 + 1) * P, :],
                                O3[:, ksl],
                            )


    nc.compile()
    return nc


def _get_nc(mm_relaxed=MM_RELAXED):
    key = ("nc", mm_relaxed)
    if key not in _CACHE:
        _CACHE[key] = _build_nc(mm_relaxed)
    return _CACHE[key]


def kernel(C, Q, w4C, w4Q, w4mlu, bias=None, trace=False, **_ignored):
    _ensure_path()
    from concourse.bass_utils import run_bass_kernel_spmd

    C = np.ascontiguousarray(np.asarray(C, dtype=np.float32))
    Q = np.ascontiguousarray(np.asarray(Q, dtype=np.float32))
    w4C = np.ascontiguousarray(np.asarray(w4C, dtype=np.float32))
    w4Q = np.ascontiguousarray(np.asarray(w4Q, dtype=np.float32))
    w4mlu = np.ascontiguousarray(np.asarray(w4mlu, dtype=np.float32))

    nc = _get_nc()
    in_maps = []
    for i in range(NCORES):
        bsl = slice(i * BPC, (i + 1) * BPC)
        in_maps.append({
            "C": np.ascontiguousarray(C[:, bsl, :]),
            "Q": np.ascontiguousarray(Q[:, bsl, :]),
            "w4C": w4C,
            "w4Q": w4Q,
            "w4mlu": w4mlu,
        })
    res = run_bass_kernel_spmd(nc, in_maps, core_ids=list(range(NCORES)),
                               trace=trace)
    _CACHE["last_result"] = res
    outs = [res.results[i]["out"] for i in range(NCORES)]
    return np.concatenate(outs, axis=0)



# revision 11
# speedup vs baseline: 1.0859x; 1.0859x over previous
"""CQAttention (trilinear context-query attention) Bass kernel for TRN2.

Full-input contract: kernel(**inputs) takes the unsharded tensors
  C (1024, 64, 256), Q (512, 64, 256), w4C (256,1), w4Q (256,1),
  w4mlu (1,1,256), bias (1,)
and returns out (64, 1024, 1024) fp32, matching the reference

  C,Q -> batch-major; S = C@w4C + (Q@w4Q)^T + (C*w4mlu)@Q^T + bias
  S1 = softmax_q(S); S2 = softmax_c(S)
  A = S1@Q ; B = (S1@S2^T)@C
  out = concat([C, A, C*A, C*B], -1) transposed to (B, 4D, Lc)

Sharding: data-parallel over batch, 8 batch items per NeuronCore.

Algebra used on-chip (per batch item):
  * bias cancels in both softmaxes (constant shift) -> dropped.
  * e0 = exp(C@w4C), e1 = exp(Q@w4Q), X = exp((C*w4mlu)@Q^T) so that
    exp(S) = e0[c] * X[c,q] * e1[q].
  * S1 = diag(1/u) X diag(e1),  u  = X @ e1            (e0 cancels)
  * S2 = diag(e0) X diag(1/v),  v  = X^T @ e0          (e1 cancels)
  * Xe := X^T with e1 folded in during the transpose evacuation, so
    A^T   = Q^T(as lhsT) @ Xe * (1/u)-broadcast
    P2    = X^T @ [Ce | e0]; H2 = P2[:, :D] / P2[:, D]  (v folded)
    B^T   = H2(as lhsT) @ Xe * (1/u)-broadcast
  Everything is computed transposed ([feature, context] layout) so output
  DMA rows are contiguous in DRAM.

Cost-model/scheduling notes (TimelineSim):
  * matmul cost = out_free_size * pe_cycle * cyc_per_row; cyc_per_row is
    keyed on ins[0] = the MOVING (rhs) operand. fp32r >= 256 wide: 1.0.
  * transpose cost keys on the identity (rhs) dtype: f32r identity ->
    1.5 cyc/row (vs 2.0 for f32); transposes execute as exact
    permutations. (A bf16 identity would be 1.0 but neuronxcc rejects
    mixed 32/16-bit matmul inputs, NCC_IBIR034.)
  * X^T is a PE-transpose of X (32 x 53ns) instead of a second scores
    matmul (16 x 213ns).
  * sub0/sub1 matvecs are N=1 matmuls (~2ns each) instead of
    gpsimd-mult + DVE tensor_reduce.
  * software pipelining: batch b+1's transpose head (CT/QT/matvecs/Ce)
    is emitted between batch b's P1T/P3T psum groups so the PE never
    waits on transpose-evacuation engines; E0T transpose groups are
    interleaved with P2 matmul chunks for the same reason.
"""

import numpy as np

LC, LQ, B, D = 1024, 512, 64, 256
NCORES = 8
BPC = B // NCORES  # batch items per core
P = 128
MC = LC // P  # 8 context chunks
TQ = LQ // P  # 4 query chunks
KD = D // P   # 2 feature chunks

MM_RELAXED = True

_CACHE = {}


def _ensure_path():
    import sys
    for p in ("/opt/trn_rl_repo",):
        if p not in sys.path:
            sys.path.insert(0, p)


def _build_nc(mm_relaxed=MM_RELAXED):
    _ensure_path()
    import concourse.bass as bass
    import concourse.bacc as bacc
    import concourse.mybir as mybir
    from concourse import tile, masks

    f32 = mybir.dt.float32
    bf16 = mybir.dt.bfloat16
    mmdt = mybir.dt.float32r if mm_relaxed else f32
    Exp = mybir.ActivationFunctionType.Exp
    Copy = mybir.ActivationFunctionType.Copy
    mult = mybir.AluOpType.mult
    add = mybir.AluOpType.add

    def r(ap):
        return ap.bitcast(mmdt)

    nc = bacc.Bacc()
    C_d = nc.dram_tensor("C", [LC, BPC, D], f32, kind="ExternalInput")
    Q_d = nc.dram_tensor("Q", [LQ, BPC, D], f32, kind="ExternalInput")
    w4C_d = nc.dram_tensor("w4C", [D, 1], f32, kind="ExternalInput")
    w4Q_d = nc.dram_tensor("w4Q", [D, 1], f32, kind="ExternalInput")
    w4mlu_d = nc.dram_tensor("w4mlu", [1, 1, D], f32, kind="ExternalInput")
    out_d = nc.dram_tensor("out", [BPC, 4 * D, LC], f32, kind="ExternalOutput")

    with tile.TileContext(nc) as tc:
        import contextlib

        with contextlib.ExitStack() as ctx:
            ep = ctx.enter_context

            consts = ep(tc.tile_pool(name="consts", bufs=1))
            cn_pool = ep(tc.tile_pool(name="cn", bufs=2))
            qn_pool = ep(tc.tile_pool(name="qn", bufs=2))
            ct_pool = ep(tc.tile_pool(name="ct", bufs=2))
            qt_pool = ep(tc.tile_pool(name="qt", bufs=2))
            qmt_pool = ep(tc.tile_pool(name="qmt", bufs=2))
            qe_pool = ep(tc.tile_pool(name="qe", bufs=2))
            ce_pool = ep(tc.tile_pool(name="ce", bufs=2))
            e0_pool = ep(tc.tile_pool(name="e0p", bufs=1))
            e0t_pool = ep(tc.tile_pool(name="e0tp", bufs=2))
            h2_pool = ep(tc.tile_pool(name="h2", bufs=2))
            rsbr_pool = ep(tc.tile_pool(name="rsbr", bufs=2))
            at_pool = ep(tc.tile_pool(name="at", bufs=2))
            bt_pool = ep(tc.tile_pool(name="bt", bufs=2))
            # O2 reuses ce_pool slots (Ce dead after P2); O3 reuses e0t slots
            o2_pool = ce_pool
            o3_pool = e0t_pool
            small_pool = ep(tc.tile_pool(name="small", bufs=2))
            row_pool = ep(tc.tile_pool(name="rows", bufs=1))

            psA = ep(tc.tile_pool(name="psA", bufs=4, space="PSUM"))
            psB = ep(tc.tile_pool(name="psB", bufs=2, space="PSUM"))
            psRow = ep(tc.tile_pool(name="psRow", bufs=2, space="PSUM"))

            # ---- per-core constants ----
            ident = consts.tile([P, P], f32)
            masks.make_identity(nc, ident[:])
            identr = consts.tile([P, P], f32)
            nc.scalar.copy(r(identr[:]), ident[:])
            ones_row = consts.tile([1, P], f32)
            nc.vector.memset(ones_row[:], 1.0)
            ones_r = consts.tile([1, P], f32)
            nc.scalar.copy(r(ones_r[:]), ones_row[:])
            w4mlu_pp = consts.tile([P, KD], f32)
            nc.sync.dma_start(
                w4mlu_pp[:], w4mlu_d[0, 0, :].rearrange("(k p) -> p k", p=P)
            )
            # matvec weight chunks duplicated into column pairs so the
            # N=1 matvec matmuls get 8-byte-aligned 2-wide PSUM outputs
            w4Cp_s = consts.tile([P, KD], f32)
            nc.sync.dma_start(
                w4Cp_s[:], w4C_d[:, 0].rearrange("(k p) -> p k", p=P)
            )
            w4Cp = consts.tile([P, 2 * KD], f32)
            for k in range(KD):
                nc.scalar.copy(
                    r(w4Cp[:, 2 * k:2 * k + 2]),
                    w4Cp_s[:, k:k + 1].broadcast_to([P, 2]),
                )
            w4Qp_s = consts.tile([P, KD], f32)
            nc.sync.dma_start(
                w4Qp_s[:], w4Q_d[:, 0].rearrange("(k p) -> p k", p=P)
            )
            w4Qp = consts.tile([P, 2 * KD], f32)
            for k in range(KD):
                nc.scalar.copy(
                    r(w4Qp[:, 2 * k:2 * k + 2]),
                    w4Qp_s[:, k:k + 1].broadcast_to([P, 2]),
                )

            load_state = {}
            head_state = {}

            def _loads(b):
                # DMA loads (natural layouts) for batch b
                Cn = cn_pool.tile([P, MC * D], f32, tag="cn")
                for m in range(MC):
                    nc.sync.dma_start(
                        Cn[:, m * D:(m + 1) * D], C_d[m * P:(m + 1) * P, b, :]
                    )
                Qn = qn_pool.tile([P, TQ * D], f32, tag="qn")
                for t in range(TQ):
                    nc.sync.dma_start(
                        Qn[:, t * D:(t + 1) * D], Q_d[t * P:(t + 1) * P, b, :]
                    )
                load_state[b] = (Cn, Qn)

            def _head_steps(b):
                """Generator of head-phase emission steps for batch b:
                transposes CT/QT (bf16 ident), matvec matmuls, exps, QmT, Ce.
                Yields after each PE psum group so the caller can interleave
                these between other psum-heavy PE work."""
                Cn, Qn = load_state.pop(b)
                CT = ct_pool.tile([P, KD * LC], f32, tag="ct")
                QT = qt_pool.tile([P, KD * LQ], f32, tag="qt")
                sub0ps = psRow.tile([P, 2 * MC], f32, tag="psRow")
                sub1ps = psRow.tile([P, 2 * TQ], f32, tag="psRow")

                # QT groups first (QmT unblocks E0 of next batch)
                for k in range(KD):
                    pst = psA.tile([P, 4 * P], f32, tag="psA")
                    for t in range(TQ):
                        nc.tensor.transpose(
                            pst[:, t * P:(t + 1) * P],
                            Qn[:, t * D + k * P: t * D + (k + 1) * P],
                            ident[:],
                        )
                    nc.scalar.copy(r(QT[:, k * LQ: k * LQ + 4 * P]), pst[:])
                    yield
                for t in range(TQ):
                    for k in range(KD):
                        nc.tensor.matmul(
                            sub1ps[:, 2 * t: 2 * t + 2],
                            r(QT[:, k * LQ + t * P: k * LQ + (t + 1) * P]),
                            r(w4Qp[:, 2 * k: 2 * k + 2]),
                            start=(k == 0),
                            stop=(k == KD - 1),
                        )
                QmT = qmt_pool.tile([P, KD * LQ], f32, tag="qmt")
                for k in range(KD):
                    nc.vector.tensor_scalar_mul(
                        r(QmT[:, k * LQ:(k + 1) * LQ]),
                        QT[:, k * LQ:(k + 1) * LQ],
                        w4mlu_pp[:, k:k + 1],
                    )
                yield

                for mg in range(0, MC, 4):
                    for k in range(KD):
                        pst = psA.tile([P, 4 * P], f32, tag="psA")
                        for j in range(4):
                            m = mg + j
                            nc.tensor.transpose(
                                pst[:, j * P:(j + 1) * P],
                                Cn[:, m * D + k * P: m * D + (k + 1) * P],
                                ident[:],
                            )
                        nc.scalar.copy(
                            r(CT[:, k * LC + mg * P: k * LC + (mg + 4) * P]),
                            pst[:],
                        )
                        yield
                    for m in range(mg, mg + 4):
                        for k in range(KD):
                            nc.tensor.matmul(
                                sub0ps[:, 2 * m: 2 * m + 2],
                                r(CT[:, k * LC + m * P: k * LC + (m + 1) * P]),
                                r(w4Cp[:, 2 * k: 2 * k + 2]),
                                start=(k == 0),
                                stop=(k == KD - 1),
                            )
                # early output of C^T rows
                for k in range(KD):
                    nc.sync.dma_start(
                        out_d[b, k * P:(k + 1) * P, :], CT[:, k * LC:(k + 1) * LC]
                    )
                yield

                e1 = small_pool.tile([P, TQ], f32, tag="e1")
                nc.scalar.activation(r(e1[:]), sub1ps[:, 0:2 * TQ:2], Exp)
                e0 = small_pool.tile([P, MC], f32, tag="e0")
                nc.scalar.activation(r(e0[:]), sub0ps[:, 0:2 * MC:2], Exp)
                yield

                Qe = qe_pool.tile([P, TQ * D], f32, tag="qe")
                for t in range(TQ):
                    nc.vector.tensor_scalar_mul(
                        r(Qe[:, t * D:(t + 1) * D]), Qn[:, t * D:(t + 1) * D],
                        e1[:, t:t + 1],
                    )
                yield

                # Ce = C * e0 with e0 appended (cols D..D+1)
                DA = D + 2
                Ce = ce_pool.tile([P, MC * DA], f32, tag="ceA")
                for m in range(MC):
                    nc.vector.tensor_scalar_mul(
                        r(Ce[:, m * DA:m * DA + D]), Cn[:, m * D:(m + 1) * D],
                        e0[:, m:m + 1],
                    )
                    nc.vector.tensor_copy(
                        r(Ce[:, m * DA + D:m * DA + DA]),
                        e0[:, m:m + 1].broadcast_to([P, 2]),
                    )
                head_state[b] = (CT, QT, QmT, Ce, Qe, e0, e1)
                yield

            def _drain(gen):
                for _ in gen:
                    pass

            def _main(b, tail_gen):
                """Main phase for batch b; emits steps from tail_gen (the
                head of batch b+1) between its own psum groups."""
                CT, QT, QmT, Ce, Qe, e0, e1 = head_state.pop(b)
                DA = D + 2

                def _tail_step():
                    if tail_gen is not None:
                        next(tail_gen, None)

                # ---- X = exp((C*w)@Q^T) [c,(m,q)] ----
                E0 = e0_pool.tile([P, MC * LQ], f32, tag="e0")
                for m in range(MC):
                    ps = psA.tile([P, LQ], f32, tag="psA")
                    for k in range(KD):
                        nc.tensor.matmul(
                            ps[:],
                            r(CT[:, k * LC + m * P: k * LC + (m + 1) * P]),
                            r(QmT[:, k * LQ:(k + 1) * LQ]),
                            start=(k == 0),
                            stop=(k == KD - 1),
                        )
                    nc.scalar.activation(r(E0[:, m * LQ:(m + 1) * LQ]), ps[:], Exp)

                # ---- Xe = X^T * e1 via PE transposes, interleaved with
                # ---- P2 = X^T @ [Ce|e0] ; H2 = P2/(e1*v)  [q,(t,d)] ----
                E0T = e0t_pool.tile([P, TQ * LC], f32, tag="e0tA")
                H2 = h2_pool.tile([P, TQ * D], f32, tag="h2")
                rec_cse = small_pool.tile([P, TQ], f32, tag="rec")
                dq = small_pool.tile([P, TQ], f32, tag="dq")

                def _e0t_group(t, mg):
                    pst = psA.tile([P, 4 * P], f32, tag="psA")
                    for j in range(4):
                        m = mg + j
                        nc.tensor.transpose(
                            r(pst[:, j * P:(j + 1) * P]),
                            r(E0[:, m * LQ + t * P: m * LQ + (t + 1) * P]),
                            r(identr[:]),
                        )
                    osl = slice(t * LC + mg * P, t * LC + (mg + 4) * P)
                    if t % 2 == 0:
                        nc.scalar.copy(r(E0T[:, osl]), pst[:])
                    else:
                        nc.vector.tensor_copy(r(E0T[:, osl]), pst[:])

                def _p2_group(qm):
                    ps = psB.tile([P, 512], f32, tag="psB")
                    for m in range(MC):
                        nc.tensor.matmul(
                            ps[:, 0:DA],
                            r(E0[:, m * LQ + qm * P: m * LQ + (qm + 1) * P]),
                            r(Ce[:, m * DA:(m + 1) * DA]),
                            start=(m == 0),
                            stop=(m == MC - 1),
                        )
                    nc.vector.reciprocal(rec_cse[:, qm:qm + 1], ps[:, D:D + 1])
                    nc.vector.tensor_tensor(
                        dq[:, qm:qm + 1], rec_cse[:, qm:qm + 1], e1[:, qm:qm + 1],
                        mult,
                    )
                    nc.scalar.activation(
                        r(H2[:, qm * D:(qm + 1) * D]), ps[:, 0:D], Copy,
                        scale=dq[:, qm:qm + 1],
                    )

                for t in range(TQ):
                    _e0t_group(t, 0)
                    _e0t_group(t, 4)
                    _p2_group(t)

                # ---- rs = colsum(Xe) as a row; rsbr = 1/rs replicated ----
                rs_row = row_pool.tile([1, LC], f32, tag="rowA")
                for n in range(2):
                    psr = psRow.tile([1, 512], f32, tag="psRow")
                    for t in range(TQ):
                        nc.tensor.matmul(
                            psr[:],
                            r(e1[:, t:t + 1]),
                            r(E0T[:, t * LC + n * 512: t * LC + (n + 1) * 512]),
                            start=(t == 0),
                            stop=(t == TQ - 1),
                        )
                    nc.scalar.copy(r(rs_row[:, n * 512:(n + 1) * 512]), psr[:])
                rsbr = rsbr_pool.tile([P, LC], f32, tag="rsbr")
                for n in range(2):
                    ps = psB.tile([P, 512], f32, tag="psB")
                    nc.tensor.matmul(
                        ps[:], r(ones_r[:]), r(rs_row[:, n * 512:(n + 1) * 512])
                    )
                    nc.vector.reciprocal(rsbr[:, n * 512:(n + 1) * 512], ps[:])

                # ---- A^T = Q(lhsT) @ Xe ; O2 = CT*AT ----
                AT = at_pool.tile([P, KD * LC], f32, tag="at")
                O2 = o2_pool.tile([P, KD * LC], f32, tag="ceA")
                for m2 in range(KD):
                    for n in range(2):
                        ps = psB.tile([P, 512], f32, tag="psB")
                        for t in range(TQ):
                            nc.tensor.matmul(
                                ps[:],
                                r(Qe[:, t * D + m2 * P: t * D + (m2 + 1) * P]),
                                r(E0T[:, t * LC + n * 512: t * LC + (n + 1) * 512]),
                                start=(t == 0),
                                stop=(t == TQ - 1),
                            )
                        sl = slice(m2 * LC + n * 512, m2 * LC + (n + 1) * 512)
                        nsl = slice(n * 512, (n + 1) * 512)
                        nc.vector.tensor_tensor(AT[:, sl], ps[:], rsbr[:, nsl], mult)
                        nc.gpsimd.tensor_tensor(O2[:, sl], CT[:, sl], AT[:, sl], mult)
                        if n == 1:
                            ksl = slice(m2 * LC, (m2 + 1) * LC)
                            nc.sync.dma_start(
                                out_d[b, 2 * P + m2 * P: 2 * P + (m2 + 1) * P, :],
                                AT[:, ksl],
                            )
                            nc.sync.dma_start(
                                out_d[b, 4 * P + m2 * P: 4 * P + (m2 + 1) * P, :],
                                O2[:, ksl],
                            )
                        _tail_step()
                        _tail_step()

                # ---- B^T = H2(lhsT) @ Xe ; O3 = CT*BT ----
                BT = bt_pool.tile([P, KD * LC], f32, tag="bt")
                O3 = o3_pool.tile([P, KD * LC], f32, tag="e0tA")
                for m2 in range(KD):
                    for n in range(2):
                        ps = psB.tile([P, 512], f32, tag="psB")
                        for t in range(TQ):
                            nc.tensor.matmul(
                                ps[:],
                                r(H2[:, t * D + m2 * P: t * D + (m2 + 1) * P]),
                                r(E0T[:, t * LC + n * 512: t * LC + (n + 1) * 512]),
                                start=(t == 0),
                                stop=(t == TQ - 1),
                            )
                        sl = slice(m2 * LC + n * 512, m2 * LC + (n + 1) * 512)
                        nsl = slice(n * 512, (n + 1) * 512)
                        nc.vector.tensor_tensor(BT[:, sl], ps[:], rsbr[:, nsl], mult)
                        nc.gpsimd.tensor_tensor(O3[:, sl], CT[:, sl], BT[:, sl], mult)
                        if n == 1:
                            ksl = slice(m2 * LC, (m2 + 1) * LC)
                            nc.sync.dma_start(
                                out_d[b, 6 * P + m2 * P: 6 * P + (m2 + 1) * P, :],
                                O3[:, ksl],
                            )
                        _tail_step()
                        _tail_step()

                # finish any remaining head steps of b+1
                if tail_gen is not None:
                    _drain(tail_gen)

            _loads(0)
            _drain(_head_steps(0))
            for b in range(BPC):
                if b + 1 < BPC:
                    _loads(b + 1)
                    _main(b, _head_steps(b + 1))
                else:
                    _main(b, None)

    nc.compile()
    return nc


def _get_nc(mm_relaxed=MM_RELAXED):
    key = ("nc", mm_relaxed)
    if key not in _CACHE:
        _CACHE[key] = _build_nc(mm_relaxed)
    return _CACHE[key]


def kernel(C, Q, w4C, w4Q, w4mlu, bias=None, trace=False, **_ignored):
    _ensure_path()
    from concourse.bass_utils import run_bass_kernel_spmd

    C = np.ascontiguousarray(np.asarray(C, dtype=np.float32))
    Q = np.ascontiguousarray(np.asarray(Q, dtype=np.float32))
    w4C = np.ascontiguousarray(np.asarray(w4C, dtype=np.float32))
    w4Q = np.ascontiguousarray(np.asarray(w4Q, dtype=np.float32))
    w4mlu = np.ascontiguousarray(np.asarray(w4mlu, dtype=np.float32))

    nc = _get_nc()
    in_maps = []
    for i in range(NCORES):
        bsl = slice(i * BPC, (i + 1) * BPC)
        in_maps.append({
            "C": np.ascontiguousarray(C[:, bsl, :]),
            "Q": np.ascontiguousarray(Q[:, bsl, :]),
            "w4C": w4C,
            "w4Q": w4Q,
            "w4mlu": w4mlu,
        })
    res = run_bass_kernel_spmd(nc, in_maps, core_ids=list(range(NCORES)),
                               trace=trace)
    _CACHE["last_result"] = res
    outs = [res.results[i]["out"] for i in range(NCORES)]
    return np.concatenate(outs, axis=0)


# revision 12
# speedup vs baseline: 1.1267x; 1.0376x over previous
"""CQAttention (trilinear context-query attention) Bass kernel for TRN2.

Full-input contract: kernel(**inputs) takes the unsharded tensors
  C (1024, 64, 256), Q (512, 64, 256), w4C (256,1), w4Q (256,1),
  w4mlu (1,1,256), bias (1,)
and returns out (64, 1024, 1024) fp32, matching the reference

  C,Q -> batch-major; S = C@w4C + (Q@w4Q)^T + (C*w4mlu)@Q^T + bias
  S1 = softmax_q(S); S2 = softmax_c(S)
  A = S1@Q ; B = (S1@S2^T)@C
  out = concat([C, A, C*A, C*B], -1) transposed to (B, 4D, Lc)

Sharding: data-parallel over batch, 8 batch items per NeuronCore.

Algebra used on-chip (per batch item):
  * bias cancels in both softmaxes (constant shift) -> dropped.
  * e0 = exp(C@w4C), e1 = exp(Q@w4Q), X = exp((C*w4mlu)@Q^T) so that
    exp(S) = e0[c] * X[c,q] * e1[q].
  * S1 = diag(1/u) X diag(e1),  u  = X @ e1            (e0 cancels)
  * S2 = diag(e0) X diag(1/v),  v  = X^T @ e0          (e1 cancels)
  * Xe := X^T with e1 folded in during the transpose evacuation, so
    A^T   = Q^T(as lhsT) @ Xe * (1/u)-broadcast
    P2    = X^T @ [Ce | e0]; H2 = P2[:, :D] / P2[:, D]  (v folded)
    B^T   = H2(as lhsT) @ Xe * (1/u)-broadcast
  Everything is computed transposed ([feature, context] layout) so output
  DMA rows are contiguous in DRAM.

Cost-model/scheduling notes (TimelineSim):
  * matmul cost = out_free_size * pe_cycle * cyc_per_row; cyc_per_row is
    keyed on ins[0] = the MOVING (rhs) operand. fp32r >= 256 wide: 1.0.
  * transpose cost keys on the identity (rhs) dtype: f32r identity ->
    1.5 cyc/row (vs 2.0 for f32); transposes execute as exact
    permutations. (A bf16 identity would be 1.0 but neuronxcc rejects
    mixed 32/16-bit matmul inputs, NCC_IBIR034.)
  * X^T is a PE-transpose of X (32 x 53ns) instead of a second scores
    matmul (16 x 213ns).
  * sub0/sub1 matvecs are N=1 matmuls (~2ns each) instead of
    gpsimd-mult + DVE tensor_reduce.
  * software pipelining: batch b+1's transpose head (CT/QT/matvecs/Ce)
    is emitted between batch b's P1T/P3T psum groups so the PE never
    waits on transpose-evacuation engines; E0T transpose groups are
    interleaved with P2 matmul chunks for the same reason.
"""

import numpy as np

LC, LQ, B, D = 1024, 512, 64, 256
NCORES = 8
BPC = B // NCORES  # batch items per core
P = 128
MC = LC // P  # 8 context chunks
TQ = LQ // P  # 4 query chunks
KD = D // P   # 2 feature chunks

MM_RELAXED = True

_CACHE = {}


def _ensure_path():
    import sys
    for p in ("/opt/trn_rl_repo",):
        if p not in sys.path:
            sys.path.insert(0, p)


def _build_nc(mm_relaxed=MM_RELAXED):
    _ensure_path()
    import concourse.bass as bass
    import concourse.bacc as bacc
    import concourse.mybir as mybir
    from concourse import tile, masks

    f32 = mybir.dt.float32
    bf16 = mybir.dt.bfloat16
    mmdt = mybir.dt.float32r if mm_relaxed else f32
    Exp = mybir.ActivationFunctionType.Exp
    Copy = mybir.ActivationFunctionType.Copy
    mult = mybir.AluOpType.mult
    add = mybir.AluOpType.add

    def r(ap):
        return ap.bitcast(mmdt)

    nc = bacc.Bacc()
    C_d = nc.dram_tensor("C", [LC, BPC, D], f32, kind="ExternalInput")
    Q_d = nc.dram_tensor("Q", [LQ, BPC, D], f32, kind="ExternalInput")
    w4C_d = nc.dram_tensor("w4C", [D, 1], f32, kind="ExternalInput")
    w4Q_d = nc.dram_tensor("w4Q", [D, 1], f32, kind="ExternalInput")
    w4mlu_d = nc.dram_tensor("w4mlu", [1, 1, D], f32, kind="ExternalInput")
    out_d = nc.dram_tensor("out", [BPC, 4 * D, LC], f32, kind="ExternalOutput")

    with tile.TileContext(nc) as tc:
        import contextlib

        with contextlib.ExitStack() as ctx:
            ep = ctx.enter_context

            consts = ep(tc.tile_pool(name="consts", bufs=1))
            cn_pool = ep(tc.tile_pool(name="cn", bufs=2))
            qn_pool = ep(tc.tile_pool(name="qn", bufs=2))
            ct_pool = ep(tc.tile_pool(name="ct", bufs=2))
            qt_pool = ep(tc.tile_pool(name="qt", bufs=2))
            qmt_pool = ep(tc.tile_pool(name="qmt", bufs=2))
            qe_pool = ep(tc.tile_pool(name="qe", bufs=2))
            e1b_pool = ep(tc.tile_pool(name="e1b", bufs=2))
            ce_pool = ep(tc.tile_pool(name="ce", bufs=2))
            e0_pool = ep(tc.tile_pool(name="e0p", bufs=1))
            e0t_pool = ep(tc.tile_pool(name="e0tp", bufs=2))
            h2_pool = ep(tc.tile_pool(name="h2", bufs=2))
            rsbr_pool = ep(tc.tile_pool(name="rsbr", bufs=2))
            at_pool = ep(tc.tile_pool(name="at", bufs=2))
            bt_pool = ep(tc.tile_pool(name="bt", bufs=2))
            # O2 reuses ce_pool slots (Ce dead after P2); O3 reuses e0t slots
            o2_pool = ce_pool
            o3_pool = e0t_pool
            small_pool = ep(tc.tile_pool(name="small", bufs=2))

            psA = ep(tc.tile_pool(name="psA", bufs=4, space="PSUM"))
            psB = ep(tc.tile_pool(name="psB", bufs=2, space="PSUM"))
            psRow = ep(tc.tile_pool(name="psRow", bufs=2, space="PSUM"))

            # ---- per-core constants ----
            ident = consts.tile([P, P], f32)
            masks.make_identity(nc, ident[:])
            identr = consts.tile([P, P], f32)
            nc.scalar.copy(r(identr[:]), ident[:])
            w4mlu_pp = consts.tile([P, KD], f32)
            nc.sync.dma_start(
                w4mlu_pp[:], w4mlu_d[0, 0, :].rearrange("(k p) -> p k", p=P)
            )
            # matvec weight chunks duplicated into column pairs so the
            # N=1 matvec matmuls get 8-byte-aligned 2-wide PSUM outputs
            w4Cp_s = consts.tile([P, KD], f32)
            nc.sync.dma_start(
                w4Cp_s[:], w4C_d[:, 0].rearrange("(k p) -> p k", p=P)
            )
            w4Cp = consts.tile([P, 2 * KD], f32)
            for k in range(KD):
                nc.scalar.copy(
                    r(w4Cp[:, 2 * k:2 * k + 2]),
                    w4Cp_s[:, k:k + 1].broadcast_to([P, 2]),
                )
            w4Qp_s = consts.tile([P, KD], f32)
            nc.sync.dma_start(
                w4Qp_s[:], w4Q_d[:, 0].rearrange("(k p) -> p k", p=P)
            )
            w4Qp = consts.tile([P, 2 * KD], f32)
            for k in range(KD):
                nc.scalar.copy(
                    r(w4Qp[:, 2 * k:2 * k + 2]),
                    w4Qp_s[:, k:k + 1].broadcast_to([P, 2]),
                )

            load_state = {}
            head_state = {}

            def _loads(b):
                # DMA loads (natural layouts) for batch b
                Qn = qn_pool.tile([P, TQ * D], f32, tag="qn")
                for t in range(TQ):
                    nc.sync.dma_start(
                        Qn[:, t * D:(t + 1) * D], Q_d[t * P:(t + 1) * P, b, :]
                    )
                Cn = cn_pool.tile([P, MC * D], f32, tag="cn")
                for m in range(MC):
                    nc.sync.dma_start(
                        Cn[:, m * D:(m + 1) * D], C_d[m * P:(m + 1) * P, b, :]
                    )
                load_state[b] = (Cn, Qn)

            def _head_steps(b):
                """Generator of head-phase emission steps for batch b:
                transposes CT/QT (bf16 ident), matvec matmuls, exps, QmT, Ce.
                Yields after each PE psum group so the caller can interleave
                these between other psum-heavy PE work."""
                Cn, Qn = load_state.pop(b)
                CT = ct_pool.tile([P, KD * LC], f32, tag="ct")
                QT = qt_pool.tile([P, KD * LQ], f32, tag="qt")
                sub0ps = psRow.tile([P, 2 * MC], f32, tag="psRow")
                sub1ps = psRow.tile([P, 2 * TQ], f32, tag="psRow")

                # QT groups first (QmT unblocks E0 of next batch)
                for k in range(KD):
                    pst = psA.tile([P, 4 * P], f32, tag="psA")
                    for t in range(TQ):
                        nc.tensor.transpose(
                            pst[:, t * P:(t + 1) * P],
                            Qn[:, t * D + k * P: t * D + (k + 1) * P],
                            ident[:],
                        )
                    nc.scalar.copy(r(QT[:, k * LQ: k * LQ + 4 * P]), pst[:])
                    yield
                for t in range(TQ):
                    for k in range(KD):
                        nc.tensor.matmul(
                            sub1ps[:, 2 * t: 2 * t + 2],
                            r(QT[:, k * LQ + t * P: k * LQ + (t + 1) * P]),
                            r(w4Qp[:, 2 * k: 2 * k + 2]),
                            start=(k == 0),
                            stop=(k == KD - 1),
                        )
                QmT = qmt_pool.tile([P, KD * LQ], f32, tag="qmt")
                for k in range(KD):
                    nc.vector.tensor_scalar_mul(
                        r(QmT[:, k * LQ:(k + 1) * LQ]),
                        QT[:, k * LQ:(k + 1) * LQ],
                        w4mlu_pp[:, k:k + 1],
                    )
                yield

                for mg in range(0, MC, 4):
                    for k in range(KD):
                        pst = psA.tile([P, 4 * P], f32, tag="psA")
                        for j in range(4):
                            m = mg + j
                            nc.tensor.transpose(
                                pst[:, j * P:(j + 1) * P],
                                Cn[:, m * D + k * P: m * D + (k + 1) * P],
                                ident[:],
                            )
                        nc.scalar.copy(
                            r(CT[:, k * LC + mg * P: k * LC + (mg + 4) * P]),
                            pst[:],
                        )
                        yield
                    for m in range(mg, mg + 4):
                        for k in range(KD):
                            nc.tensor.matmul(
                                sub0ps[:, 2 * m: 2 * m + 2],
                                r(CT[:, k * LC + m * P: k * LC + (m + 1) * P]),
                                r(w4Cp[:, 2 * k: 2 * k + 2]),
                                start=(k == 0),
                                stop=(k == KD - 1),
                            )
                # early output of C^T rows
                for k in range(KD):
                    nc.sync.dma_start(
                        out_d[b, k * P:(k + 1) * P, :], CT[:, k * LC:(k + 1) * LC]
                    )
                yield

                e1 = small_pool.tile([P, TQ], f32, tag="e1")
                nc.scalar.activation(r(e1[:]), sub1ps[:, 0:2 * TQ:2], Exp)
                e0 = small_pool.tile([P, MC], f32, tag="e0")
                nc.scalar.activation(r(e0[:]), sub0ps[:, 0:2 * MC:2], Exp)
                yield

                Qe = qe_pool.tile([P, TQ * D], f32, tag="qe")
                for t in range(TQ):
                    nc.vector.tensor_scalar_mul(
                        r(Qe[:, t * D:(t + 1) * D]), Qn[:, t * D:(t + 1) * D],
                        e1[:, t:t + 1],
                    )
                # e1 replicated across free dim: lhsT for the direct-rsbr
                # matmuls (rs broadcast across partitions in one step)
                E1B = e1b_pool.tile([P, TQ * P], f32, tag="e1b")
                for t in range(TQ):
                    nc.vector.tensor_copy(
                        r(E1B[:, t * P:(t + 1) * P]),
                        e1[:, t:t + 1].broadcast_to([P, P]),
                    )
                yield

                # Ce = C * e0 with e0 appended (cols D..D+1)
                DA = D + 2
                Ce = ce_pool.tile([P, MC * DA], f32, tag="ceA")
                for m in range(MC):
                    nc.vector.tensor_scalar_mul(
                        r(Ce[:, m * DA:m * DA + D]), Cn[:, m * D:(m + 1) * D],
                        e0[:, m:m + 1],
                    )
                    nc.vector.tensor_copy(
                        r(Ce[:, m * DA + D:m * DA + DA]),
                        e0[:, m:m + 1].broadcast_to([P, 2]),
                    )
                head_state[b] = (CT, QT, QmT, Ce, Qe, E1B, e0, e1)
                yield

            def _drain(gen):
                for _ in gen:
                    pass

            def _main(b, tail_gen):
                """Main phase for batch b; emits steps from tail_gen (the
                head of batch b+1) between its own psum groups."""
                CT, QT, QmT, Ce, Qe, E1B, e0, e1 = head_state.pop(b)
                DA = D + 2

                def _tail_step():
                    if tail_gen is not None:
                        next(tail_gen, None)

                # ---- X = exp((C*w)@Q^T) [c,(m,q)] ----
                E0 = e0_pool.tile([P, MC * LQ], f32, tag="e0")
                for m in range(MC):
                    ps = psA.tile([P, LQ], f32, tag="psA")
                    for k in range(KD):
                        nc.tensor.matmul(
                            ps[:],
                            r(CT[:, k * LC + m * P: k * LC + (m + 1) * P]),
                            r(QmT[:, k * LQ:(k + 1) * LQ]),
                            start=(k == 0),
                            stop=(k == KD - 1),
                        )
                    nc.scalar.activation(r(E0[:, m * LQ:(m + 1) * LQ]), ps[:], Exp)

                # ---- Xe = X^T * e1 via PE transposes, interleaved with
                # ---- P2 = X^T @ [Ce|e0] ; H2 = P2/(e1*v)  [q,(t,d)] ----
                E0T = e0t_pool.tile([P, TQ * LC], f32, tag="e0tA")
                H2 = h2_pool.tile([P, TQ * D], f32, tag="h2")
                rec_cse = small_pool.tile([P, TQ], f32, tag="rec")
                dq = small_pool.tile([P, TQ], f32, tag="dq")

                def _e0t_group(t, mg):
                    pst = psA.tile([P, 4 * P], f32, tag="psA")
                    for j in range(4):
                        m = mg + j
                        nc.tensor.transpose(
                            r(pst[:, j * P:(j + 1) * P]),
                            r(E0[:, m * LQ + t * P: m * LQ + (t + 1) * P]),
                            r(identr[:]),
                        )
                    osl = slice(t * LC + mg * P, t * LC + (mg + 4) * P)
                    if t % 2 == 0:
                        nc.scalar.copy(r(E0T[:, osl]), pst[:])
                    else:
                        nc.vector.tensor_copy(r(E0T[:, osl]), pst[:])

                def _p2_group(qm):
                    ps = psB.tile([P, 512], f32, tag="psB")
                    for m in range(MC):
                        nc.tensor.matmul(
                            ps[:, 0:DA],
                            r(E0[:, m * LQ + qm * P: m * LQ + (qm + 1) * P]),
                            r(Ce[:, m * DA:(m + 1) * DA]),
                            start=(m == 0),
                            stop=(m == MC - 1),
                        )
                    nc.vector.reciprocal(rec_cse[:, qm:qm + 1], ps[:, D:D + 1])
                    nc.vector.tensor_tensor(
                        dq[:, qm:qm + 1], rec_cse[:, qm:qm + 1], e1[:, qm:qm + 1],
                        mult,
                    )
                    nc.scalar.activation(
                        r(H2[:, qm * D:(qm + 1) * D]), ps[:, 0:D], Copy,
                        scale=dq[:, qm:qm + 1],
                    )

                for t in range(TQ):
                    _e0t_group(t, 0)
                    _e0t_group(t, 4)
                    _p2_group(t)

                # ---- rsbr = 1/(rs broadcast): E1B-lhsT puts rs on every
                # ---- partition directly ----
                rsbr = rsbr_pool.tile([P, LC], f32, tag="rsbr")
                for n in range(2):
                    ps = psB.tile([P, 512], f32, tag="psB")
                    for t in range(TQ):
                        nc.tensor.matmul(
                            ps[:],
                            r(E1B[:, t * P:(t + 1) * P]),
                            r(E0T[:, t * LC + n * 512: t * LC + (n + 1) * 512]),
                            start=(t == 0),
                            stop=(t == TQ - 1),
                        )
                    nc.vector.reciprocal(rsbr[:, n * 512:(n + 1) * 512], ps[:])

                # ---- A^T = Q(lhsT) @ Xe ; O2 = CT*AT ----
                AT = at_pool.tile([P, KD * LC], f32, tag="at")
                O2 = o2_pool.tile([P, KD * LC], f32, tag="ceA")
                for m2 in range(KD):
                    for n in range(2):
                        ps = psB.tile([P, 512], f32, tag="psB")
                        for t in range(TQ):
                            nc.tensor.matmul(
                                ps[:],
                                r(Qe[:, t * D + m2 * P: t * D + (m2 + 1) * P]),
                                r(E0T[:, t * LC + n * 512: t * LC + (n + 1) * 512]),
                                start=(t == 0),
                                stop=(t == TQ - 1),
                            )
                        sl = slice(m2 * LC + n * 512, m2 * LC + (n + 1) * 512)
                        nsl = slice(n * 512, (n + 1) * 512)
                        nc.vector.tensor_tensor(AT[:, sl], ps[:], rsbr[:, nsl], mult)
                        nc.gpsimd.tensor_tensor(O2[:, sl], CT[:, sl], AT[:, sl], mult)
                        nc.sync.dma_start(
                            out_d[b, 2 * P + m2 * P: 2 * P + (m2 + 1) * P, nsl],
                            AT[:, sl],
                        )
                        nc.sync.dma_start(
                            out_d[b, 4 * P + m2 * P: 4 * P + (m2 + 1) * P, nsl],
                            O2[:, sl],
                        )
                        _tail_step()
                        _tail_step()

                # ---- B^T = H2(lhsT) @ Xe ; O3 = CT*BT ----
                BT = bt_pool.tile([P, KD * LC], f32, tag="bt")
                O3 = o3_pool.tile([P, KD * LC], f32, tag="e0tA")
                for m2 in range(KD):
                    for n in range(2):
                        ps = psB.tile([P, 512], f32, tag="psB")
                        for t in range(TQ):
                            nc.tensor.matmul(
                                ps[:],
                                r(H2[:, t * D + m2 * P: t * D + (m2 + 1) * P]),
                                r(E0T[:, t * LC + n * 512: t * LC + (n + 1) * 512]),
                                start=(t == 0),
                                stop=(t == TQ - 1),
                            )
                        sl = slice(m2 * LC + n * 512, m2 * LC + (n + 1) * 512)
                        nsl = slice(n * 512, (n + 1) * 512)
                        nc.vector.tensor_tensor(BT[:, sl], ps[:], rsbr[:, nsl], mult)
                        nc.gpsimd.tensor_tensor(O3[:, sl], CT[:, sl], BT[:, sl], mult)
                        nc.sync.dma_start(
                            out_d[b, 6 * P + m2 * P: 6 * P + (m2 + 1) * P, nsl],
                            O3[:, sl],
                        )
                        _tail_step()
                        _tail_step()

                # finish any remaining head steps of b+1
                if tail_gen is not None:
                    _drain(tail_gen)

            _loads(0)
            _drain(_head_steps(0))
            for b in range(BPC):
                if b + 1 < BPC:
                    _loads(b + 1)
                    _main(b, _head_steps(b + 1))
                else:
                    _main(b, None)

    nc.compile()
    return nc


def _get_nc(mm_relaxed=MM_RELAXED):
    key = ("nc", mm_relaxed)
    if key not in _CACHE:
        _CACHE[key] = _build_nc(mm_relaxed)
    return _CACHE[key]


def kernel(C, Q, w4C, w4Q, w4mlu, bias=None, trace=False, **_ignored):
    _ensure_path()
    from concourse.bass_utils import run_bass_kernel_spmd

    C = np.ascontiguousarray(np.asarray(C, dtype=np.float32))
    Q = np.ascontiguousarray(np.asarray(Q, dtype=np.float32))
    w4C = np.ascontiguousarray(np.asarray(w4C, dtype=np.float32))
    w4Q = np.ascontiguousarray(np.asarray(w4Q, dtype=np.float32))
    w4mlu = np.ascontiguousarray(np.asarray(w4mlu, dtype=np.float32))

    nc = _get_nc()
    in_maps = []
    for i in range(NCORES):
        bsl = slice(i * BPC, (i + 1) * BPC)
        in_maps.append({
            "C": np.ascontiguousarray(C[:, bsl, :]),
            "Q": np.ascontiguousarray(Q[:, bsl, :]),
            "w4C": w4C,
            "w4Q": w4Q,
            "w4mlu": w4mlu,
        })
    res = run_bass_kernel_spmd(nc, in_maps, core_ids=list(range(NCORES)),
                               trace=trace)
    _CACHE["last_result"] = res
    outs = [res.results[i]["out"] for i in range(NCORES)]
    return np.concatenate(outs, axis=0)


# revision 21
# speedup vs baseline: 1.1455x; 1.0167x over previous
"""CQAttention (trilinear context-query attention) Bass kernel for TRN2.

Full-input contract: kernel(**inputs) takes the unsharded tensors
  C (1024, 64, 256), Q (512, 64, 256), w4C (256,1), w4Q (256,1),
  w4mlu (1,1,256), bias (1,)
and returns out (64, 1024, 1024) fp32, matching the reference

  C,Q -> batch-major; S = C@w4C + (Q@w4Q)^T + (C*w4mlu)@Q^T + bias
  S1 = softmax_q(S); S2 = softmax_c(S)
  A = S1@Q ; B = (S1@S2^T)@C
  out = concat([C, A, C*A, C*B], -1) transposed to (B, 4D, Lc)

Sharding: data-parallel over batch, 8 batch items per NeuronCore.

Algebra used on-chip (per batch item):
  * bias cancels in both softmaxes (constant shift) -> dropped.
  * e0 = exp(C@w4C), e1 = exp(Q@w4Q), X = exp((C*w4mlu)@Q^T) so that
    exp(S) = e0[c] * X[c,q] * e1[q].
  * S1 = diag(1/u) X diag(e1),  u  = X @ e1            (e0 cancels)
  * S2 = diag(e0) X diag(1/v),  v  = X^T @ e0          (e1 cancels)
  * Xe := X^T with e1 folded in during the transpose evacuation, so
    A^T   = Q^T(as lhsT) @ Xe * (1/u)-broadcast
    P2    = X^T @ [Ce | e0]; H2 = P2[:, :D] / P2[:, D]  (v folded)
    B^T   = H2(as lhsT) @ Xe * (1/u)-broadcast
  Everything is computed transposed ([feature, context] layout) so output
  DMA rows are contiguous in DRAM.

Cost-model/scheduling notes (TimelineSim):
  * matmul cost = out_free_size * pe_cycle * cyc_per_row; cyc_per_row is
    keyed on ins[0] = the MOVING (rhs) operand. fp32r >= 256 wide: 1.0.
  * transpose cost keys on the identity (rhs) dtype: f32r identity ->
    1.5 cyc/row (vs 2.0 for f32); transposes execute as exact
    permutations. (A bf16 identity would be 1.0 but neuronxcc rejects
    mixed 32/16-bit matmul inputs, NCC_IBIR034.)
  * X^T is a PE-transpose of X (32 x 53ns) instead of a second scores
    matmul (16 x 213ns).
  * sub0/sub1 matvecs are N=1 matmuls (~2ns each) instead of
    gpsimd-mult + DVE tensor_reduce.
  * software pipelining: batch b+1's transpose head (CT/QT/matvecs/Ce)
    is emitted between batch b's P1T/P3T psum groups so the PE never
    waits on transpose-evacuation engines; E0T transpose groups are
    interleaved with P2 matmul chunks for the same reason.
"""

import numpy as np

LC, LQ, B, D = 1024, 512, 64, 256
NCORES = 8
BPC = B // NCORES  # batch items per core
P = 128
MC = LC // P  # 8 context chunks
TQ = LQ // P  # 4 query chunks
KD = D // P   # 2 feature chunks

MM_RELAXED = True

_CACHE = {}


def _ensure_path():
    import sys
    for p in ("/opt/trn_rl_repo",):
        if p not in sys.path:
            sys.path.insert(0, p)


def _build_nc(mm_relaxed=MM_RELAXED):
    _ensure_path()
    import concourse.bass as bass
    import concourse.bacc as bacc
    import concourse.mybir as mybir
    from concourse import tile, masks

    f32 = mybir.dt.float32
    bf16 = mybir.dt.bfloat16
    mmdt = mybir.dt.float32r if mm_relaxed else f32
    Exp = mybir.ActivationFunctionType.Exp
    Copy = mybir.ActivationFunctionType.Copy
    mult = mybir.AluOpType.mult
    add = mybir.AluOpType.add

    def r(ap):
        return ap.bitcast(mmdt)

    nc = bacc.Bacc()
    C_d = nc.dram_tensor("C", [LC, BPC, D], f32, kind="ExternalInput")
    Q_d = nc.dram_tensor("Q", [LQ, BPC, D], f32, kind="ExternalInput")
    w4C_d = nc.dram_tensor("w4C", [D, 1], f32, kind="ExternalInput")
    w4Q_d = nc.dram_tensor("w4Q", [D, 1], f32, kind="ExternalInput")
    w4mlu_d = nc.dram_tensor("w4mlu", [1, 1, D], f32, kind="ExternalInput")
    out_d = nc.dram_tensor("out", [BPC, 4 * D, LC], f32, kind="ExternalOutput")

    with tile.TileContext(nc) as tc:
        import contextlib

        with contextlib.ExitStack() as ctx:
            ep = ctx.enter_context

            consts = ep(tc.tile_pool(name="consts", bufs=1))
            cn_pool = ep(tc.tile_pool(name="cn", bufs=2))
            cnr_pool = ep(tc.tile_pool(name="cnr", bufs=1))
            qn_pool = ep(tc.tile_pool(name="qn", bufs=2))
            ct_pool = ep(tc.tile_pool(name="ct", bufs=2))
            qt_pool = ep(tc.tile_pool(name="qt", bufs=2))
            qmt_pool = ep(tc.tile_pool(name="qmt", bufs=2))
            qe_pool = ep(tc.tile_pool(name="qe", bufs=2))
            e1b_pool = ep(tc.tile_pool(name="e1b", bufs=2))
            ce_pool = ep(tc.tile_pool(name="ce", bufs=2))
            e0_pool = ep(tc.tile_pool(name="e0p", bufs=1))
            e0t_pool = ep(tc.tile_pool(name="e0tp", bufs=2))
            h2_pool = ep(tc.tile_pool(name="h2", bufs=2))
            rsbr_pool = ep(tc.tile_pool(name="rsbr", bufs=2))
            at_pool = ep(tc.tile_pool(name="at", bufs=2))
            bt_pool = ep(tc.tile_pool(name="bt", bufs=2))
            # O2 reuses ce_pool slots (Ce dead after P2); O3 reuses e0t slots
            o2_pool = ce_pool
            o3_pool = e0t_pool
            small_pool = ep(tc.tile_pool(name="small", bufs=2))

            psA = ep(tc.tile_pool(name="psA", bufs=4, space="PSUM"))
            psB = ep(tc.tile_pool(name="psB", bufs=2, space="PSUM"))
            psRow = ep(tc.tile_pool(name="psRow", bufs=2, space="PSUM"))

            # ---- per-core constants ----
            ident = consts.tile([P, P], f32)
            masks.make_identity(nc, ident[:])
            identr = consts.tile([P, P], f32)
            nc.scalar.copy(r(identr[:]), ident[:])
            w4mlu_pp = consts.tile([P, KD], f32)
            nc.sync.dma_start(
                w4mlu_pp[:], w4mlu_d[0, 0, :].rearrange("(k p) -> p k", p=P)
            )
            # matvec weight chunks duplicated into column pairs so the
            # N=1 matvec matmuls get 8-byte-aligned 2-wide PSUM outputs
            w4Cp_s = consts.tile([P, KD], f32)
            nc.sync.dma_start(
                w4Cp_s[:], w4C_d[:, 0].rearrange("(k p) -> p k", p=P)
            )
            w4Cp = consts.tile([P, 2 * KD], f32)
            for k in range(KD):
                nc.scalar.copy(
                    r(w4Cp[:, 2 * k:2 * k + 2]),
                    w4Cp_s[:, k:k + 1].broadcast_to([P, 2]),
                )
            w4Qp_s = consts.tile([P, KD], f32)
            w4Qp = consts.tile([P, 2 * KD], f32)
            for k in range(KD):
                nc.scalar.copy(
                    r(w4Qp[:, 2 * k:2 * k + 2]),
                    w4Qp_s[:, k:k + 1].broadcast_to([P, 2]),
                )

            load_state = {}
            head_state = {}

            def _consts_dmas():
                nc.sync.dma_start(
                    w4mlu_pp[:], w4mlu_d[0, 0, :].rearrange("(k p) -> p k", p=P)
                )
                nc.sync.dma_start(
                    w4Cp_s[:], w4C_d[:, 0].rearrange("(k p) -> p k", p=P)
                )
                nc.sync.dma_start(
                    w4Qp_s[:], w4Q_d[:, 0].rearrange("(k p) -> p k", p=P)
                )
                for k in range(KD):
                    nc.scalar.copy(
                        r(w4Cp[:, 2 * k:2 * k + 2]),
                        w4Cp_s[:, k:k + 1].broadcast_to([P, 2]),
                    )
                    nc.scalar.copy(
                        r(w4Qp[:, 2 * k:2 * k + 2]),
                        w4Qp_s[:, k:k + 1].broadcast_to([P, 2]),
                    )

            def _loads(b, split=False):
                # DMA loads (natural layouts) for batch b; batch 0 is split
                # per-chunk so the first transposes start sooner
                Qn = qn_pool.tile([P, TQ * D], f32, tag="qn")
                if split:
                    for t in range(TQ):
                        nc.sync.dma_start(
                            Qn[:, t * D:(t + 1) * D],
                            Q_d[t * P:(t + 1) * P, b, :],
                        )
                else:
                    nc.sync.dma_start(
                        Qn[:].rearrange("p (t d) -> p t d", t=TQ),
                        Q_d[:, b, :].rearrange("(t p) d -> p t d", p=P),
                    )
                Cn = cn_pool.tile([P, MC * D], f32, tag="cn")
                if split:
                    for mg in range(0, MC, 4):
                        nc.sync.dma_start(
                            Cn[:, mg * D:(mg + 4) * D].rearrange(
                                "p (m d) -> p m d", m=4),
                            C_d[mg * P:(mg + 4) * P, b, :].rearrange(
                                "(m p) d -> p m d", p=P),
                        )
                else:
                    nc.sync.dma_start(
                        Cn[:].rearrange("p (m d) -> p m d", m=MC),
                        C_d[:, b, :].rearrange("(m p) d -> p m d", p=P),
                    )
                load_state[b] = (Cn, Qn)

            def _head_steps(b, first=False):
                """Generator of head-phase emission steps for batch b:
                transposes CT/QT (bf16 ident), matvec matmuls, exps, QmT, Ce.
                Yields after each PE psum group so the caller can interleave
                these between other psum-heavy PE work."""
                Cn, Qn = load_state.pop(b)
                CT = ct_pool.tile([P, KD * LC], f32, tag="ct")
                QT = qt_pool.tile([P, KD * LQ], f32, tag="qt")
                # f32r-rounded copy of Cn so the CT transposes run in f32r
                # mode (1.5 cyc/row instead of 2.0); batch 0 has nothing to
                # overlap the copy with, so it keeps the f32 path
                if not first:
                    Cnr = cnr_pool.tile([P, MC * D], f32, tag="cnr")
                    nc.scalar.copy(
                        r(Cnr[:, 0:MC * D // 2]), Cn[:, 0:MC * D // 2])
                    nc.vector.tensor_copy(
                        r(Cnr[:, MC * D // 2:]), Cn[:, MC * D // 2:]
                    )
                sub0ps = psRow.tile([P, 2 * MC], f32, tag="psRow")
                sub1ps = psRow.tile([P, 2 * TQ], f32, tag="psRow")

                # QT groups first (QmT unblocks E0 of next batch)
                for k in range(KD):
                    pst = psA.tile([P, 4 * P], f32, tag="psA")
                    for t in range(TQ):
                        nc.tensor.transpose(
                            pst[:, t * P:(t + 1) * P],
                            Qn[:, t * D + k * P: t * D + (k + 1) * P],
                            ident[:],
                        )
                    nc.scalar.copy(r(QT[:, k * LQ: k * LQ + 4 * P]), pst[:])
                    yield
                for t in range(TQ):
                    for k in range(KD):
                        nc.tensor.matmul(
                            sub1ps[:, 2 * t: 2 * t + 2],
                            r(QT[:, k * LQ + t * P: k * LQ + (t + 1) * P]),
                            r(w4Qp[:, 2 * k: 2 * k + 2]),
                            start=(k == 0),
                            stop=(k == KD - 1),
                        )
                QmT = qmt_pool.tile([P, KD * LQ], f32, tag="qmt")
                for k in range(KD):
                    nc.vector.tensor_scalar_mul(
                        r(QmT[:, k * LQ:(k + 1) * LQ]),
                        QT[:, k * LQ:(k + 1) * LQ],
                        w4mlu_pp[:, k:k + 1],
                    )
                yield

                for mg in range(0, MC, 4):
                    for k in range(KD):
                        pst = psA.tile([P, 4 * P], f32, tag="psA")
                        for j in range(4):
                            m = mg + j
                            if first:
                                nc.tensor.transpose(
                                    pst[:, j * P:(j + 1) * P],
                                    Cn[:, m * D + k * P: m * D + (k + 1) * P],
                                    ident[:],
                                )
                            else:
                                nc.tensor.transpose(
                                    r(pst[:, j * P:(j + 1) * P]),
                                    r(Cnr[:, m * D + k * P:
                                          m * D + (k + 1) * P]),
                                    r(identr[:]),
                                )
                        nc.scalar.copy(
                            r(CT[:, k * LC + mg * P: k * LC + (mg + 4) * P]),
                            pst[:],
                        )
                        yield
                    for m in range(mg, mg + 4):
                        for k in range(KD):
                            nc.tensor.matmul(
                                sub0ps[:, 2 * m: 2 * m + 2],
                                r(CT[:, k * LC + m * P: k * LC + (m + 1) * P]),
                                r(w4Cp[:, 2 * k: 2 * k + 2]),
                                start=(k == 0),
                                stop=(k == KD - 1),
                            )
                # early output of C^T rows (single merged DMA)
                nc.sync.dma_start(
                    out_d[b, 0:KD * P, :].rearrange("(k p) c -> p k c", p=P),
                    CT[:].rearrange("p (k c) -> p k c", k=KD),
                )
                yield

                e1 = small_pool.tile([P, TQ], f32, tag="e1")
                nc.scalar.activation(r(e1[:]), sub1ps[:, 0:2 * TQ:2], Exp)
                e0 = small_pool.tile([P, MC], f32, tag="e0")
                nc.scalar.activation(r(e0[:]), sub0ps[:, 0:2 * MC:2], Exp)
                yield

                Qe = qe_pool.tile([P, TQ * D], f32, tag="qe")
                for t in range(TQ):
                    nc.vector.tensor_scalar_mul(
                        r(Qe[:, t * D:(t + 1) * D]), Qn[:, t * D:(t + 1) * D],
                        e1[:, t:t + 1],
                    )
                # e1 replicated across free dim: lhsT for the direct-rsbr
                # matmuls (rs broadcast across partitions in one step)
                E1B = e1b_pool.tile([P, TQ * P], f32, tag="e1b")
                for t in range(TQ):
                    nc.vector.tensor_copy(
                        r(E1B[:, t * P:(t + 1) * P]),
                        e1[:, t:t + 1].broadcast_to([P, P]),
                    )
                yield

                # Ce = C * e0 with e0 appended (cols D..D+1)
                DA = D + 2
                Ce = ce_pool.tile([P, MC * DA], f32, tag="ceA")
                for m in range(MC):
                    nc.vector.tensor_scalar_mul(
                        r(Ce[:, m * DA:m * DA + D]), Cn[:, m * D:(m + 1) * D],
                        e0[:, m:m + 1],
                    )
                    nc.vector.tensor_copy(
                        r(Ce[:, m * DA + D:m * DA + DA]),
                        e0[:, m:m + 1].broadcast_to([P, 2]),
                    )
                head_state[b] = (CT, QT, QmT, Ce, Qe, E1B, e0, e1)
                yield

            def _drain(gen):
                for _ in gen:
                    pass

            def _main(b, tail_gen):
                """Main phase for batch b; emits steps from tail_gen (the
                head of batch b+1) between its own psum groups."""
                CT, QT, QmT, Ce, Qe, E1B, e0, e1 = head_state.pop(b)
                DA = D + 2

                def _tail_step():
                    if tail_gen is not None:
                        next(tail_gen, None)

                # ---- X = exp((C*w)@Q^T) [c,(m,q)] ----
                E0 = e0_pool.tile([P, MC * LQ], f32, tag="e0")
                for m in range(MC):
                    ps = psA.tile([P, LQ], f32, tag="psA")
                    for k in range(KD):
                        nc.tensor.matmul(
                            ps[:],
                            r(CT[:, k * LC + m * P: k * LC + (m + 1) * P]),
                            r(QmT[:, k * LQ:(k + 1) * LQ]),
                            start=(k == 0),
                            stop=(k == KD - 1),
                        )
                    nc.scalar.activation(r(E0[:, m * LQ:(m + 1) * LQ]), ps[:], Exp)

                # ---- Xe = X^T * e1 via PE transposes, interleaved with
                # ---- P2 = X^T @ [Ce|e0] ; H2 = P2/(e1*v)  [q,(t,d)] ----
                E0T = e0t_pool.tile([P, TQ * LC], f32, tag="e0tA")
                H2 = h2_pool.tile([P, TQ * D], f32, tag="h2")
                rec_cse = small_pool.tile([P, TQ], f32, tag="rec")
                dq = small_pool.tile([P, TQ], f32, tag="dq")

                def _e0t_group(t, mg):
                    pst = psA.tile([P, 4 * P], f32, tag="psA")
                    for j in range(4):
                        m = mg + j
                        nc.tensor.transpose(
                            r(pst[:, j * P:(j + 1) * P]),
                            r(E0[:, m * LQ + t * P: m * LQ + (t + 1) * P]),
                            r(identr[:]),
                        )
                    osl = slice(t * LC + mg * P, t * LC + (mg + 4) * P)
                    if t % 2 == 0:
                        nc.scalar.copy(r(E0T[:, osl]), pst[:])
                    else:
                        nc.vector.tensor_copy(r(E0T[:, osl]), pst[:])

                def _p2_group(qm):
                    ps = psB.tile([P, 512], f32, tag="psB")
                    for m in range(MC):
                        nc.tensor.matmul(
                            ps[:, 0:DA],
                            r(E0[:, m * LQ + qm * P: m * LQ + (qm + 1) * P]),
                            r(Ce[:, m * DA:(m + 1) * DA]),
                            start=(m == 0),
                            stop=(m == MC - 1),
                        )
                    nc.vector.reciprocal(rec_cse[:, qm:qm + 1], ps[:, D:D + 1])
                    nc.vector.tensor_tensor(
                        dq[:, qm:qm + 1], rec_cse[:, qm:qm + 1], e1[:, qm:qm + 1],
                        mult,
                    )
                    nc.scalar.activation(
                        r(H2[:, qm * D:(qm + 1) * D]), ps[:, 0:D], Copy,
                        scale=dq[:, qm:qm + 1],
                    )

                for t in range(TQ):
                    _e0t_group(t, 0)
                    _e0t_group(t, 4)
                    _p2_group(t)

                # ---- rsbr = 1/(rs broadcast): E1B-lhsT puts rs on every
                # ---- partition directly ----
                rsbr = rsbr_pool.tile([P, LC], f32, tag="rsbr")
                for n in range(2):
                    ps = psB.tile([P, 512], f32, tag="psB")
                    for t in range(TQ):
                        nc.tensor.matmul(
                            ps[:],
                            r(E1B[:, t * P:(t + 1) * P]),
                            r(E0T[:, t * LC + n * 512: t * LC + (n + 1) * 512]),
                            start=(t == 0),
                            stop=(t == TQ - 1),
                        )
                    nc.vector.reciprocal(rsbr[:, n * 512:(n + 1) * 512], ps[:])

                # ---- A^T = Q(lhsT) @ Xe ; O2 = CT*AT ----
                AT = at_pool.tile([P, KD * LC], f32, tag="at")
                O2 = o2_pool.tile([P, KD * LC], f32, tag="ceA")
                for m2 in range(KD):
                    for n in range(2):
                        ps = psB.tile([P, 512], f32, tag="psB")
                        for t in range(TQ):
                            nc.tensor.matmul(
                                ps[:],
                                r(Qe[:, t * D + m2 * P: t * D + (m2 + 1) * P]),
                                r(E0T[:, t * LC + n * 512: t * LC + (n + 1) * 512]),
                                start=(t == 0),
                                stop=(t == TQ - 1),
                            )
                        sl = slice(m2 * LC + n * 512, m2 * LC + (n + 1) * 512)
                        nsl = slice(n * 512, (n + 1) * 512)
                        nc.vector.tensor_tensor(AT[:, sl], ps[:], rsbr[:, nsl], mult)
                        nc.gpsimd.tensor_tensor(O2[:, sl], CT[:, sl], AT[:, sl], mult)
                        nc.sync.dma_start(
                            out_d[b, 2 * P + m2 * P: 2 * P + (m2 + 1) * P, nsl],
                            AT[:, sl],
                        )
                        nc.sync.dma_start(
                            out_d[b, 4 * P + m2 * P: 4 * P + (m2 + 1) * P, nsl],
                            O2[:, sl],
                        )
                        _tail_step()
                        _tail_step()

                # ---- B^T = H2(lhsT) @ Xe ; O3 = CT*BT ----
                BT = bt_pool.tile([P, KD * LC], f32, tag="bt")
                O3 = o3_pool.tile([P, KD * LC], f32, tag="e0tA")
                for m2 in range(KD):
                    for n in range(2):
                        ps = psB.tile([P, 512], f32, tag="psB")
                        for t in range(TQ):
                            nc.tensor.matmul(
                                ps[:],
                                r(H2[:, t * D + m2 * P: t * D + (m2 + 1) * P]),
                                r(E0T[:, t * LC + n * 512: t * LC + (n + 1) * 512]),
                                start=(t == 0),
                                stop=(t == TQ - 1),
                            )
                        sl = slice(m2 * LC + n * 512, m2 * LC + (n + 1) * 512)
                        nsl = slice(n * 512, (n + 1) * 512)
                        last = tail_gen is None and m2 == KD - 1 and n == 1
                        if not last:
                            nc.vector.tensor_tensor(
                                BT[:, sl], ps[:], rsbr[:, nsl], mult)
                            nc.gpsimd.tensor_tensor(
                                O3[:, sl], CT[:, sl], BT[:, sl], mult)
                            nc.sync.dma_start(
                                out_d[b, 6 * P + m2 * P: 6 * P + (m2 + 1) * P,
                                      nsl],
                                O3[:, sl],
                            )
                        else:
                            # final chunk of the final batch: quarter-split
                            # chain on DVE to shorten the drain tail
                            for q4 in range(2):
                                qsl = slice(sl.start + q4 * 256,
                                            sl.start + (q4 + 1) * 256)
                                qnsl = slice(nsl.start + q4 * 256,
                                             nsl.start + (q4 + 1) * 256)
                                nc.vector.tensor_tensor(
                                    BT[:, qsl], ps[:, q4 * 256:(q4 + 1) * 256],
                                    rsbr[:, qnsl], mult)
                                nc.vector.tensor_tensor(
                                    O3[:, qsl], CT[:, qsl], BT[:, qsl], mult)
                                nc.sync.dma_start(
                                    out_d[b, 6 * P + m2 * P:
                                          6 * P + (m2 + 1) * P, qnsl],
                                    O3[:, qsl],
                                )
                        _tail_step()
                        _tail_step()

                # finish any remaining head steps of b+1
                if tail_gen is not None:
                    _drain(tail_gen)

            _loads(0, split=True)
            _consts_dmas()
            _drain(_head_steps(0, first=True))
            for b in range(BPC):
                if b + 1 < BPC:
                    _loads(b + 1)
                    _main(b, _head_steps(b + 1))
                else:
                    _main(b, None)

    nc.compile()
    return nc


def _get_nc(mm_relaxed=MM_RELAXED):
    key = ("nc", mm_relaxed)
    if key not in _CACHE:
        _CACHE[key] = _build_nc(mm_relaxed)
    return _CACHE[key]


def kernel(C, Q, w4C, w4Q, w4mlu, bias=None, trace=False, **_ignored):
    _ensure_path()
    from concourse.bass_utils import run_bass_kernel_spmd

    C = np.ascontiguousarray(np.asarray(C, dtype=np.float32))
    Q = np.ascontiguousarray(np.asarray(Q, dtype=np.float32))
    w4C = np.ascontiguousarray(np.asarray(w4C, dtype=np.float32))
    w4Q = np.ascontiguousarray(np.asarray(w4Q, dtype=np.float32))
    w4mlu = np.ascontiguousarray(np.asarray(w4mlu, dtype=np.float32))

    nc = _get_nc()
    in_maps = []
    for i in range(NCORES):
        bsl = slice(i * BPC, (i + 1) * BPC)
        in_maps.append({
            "C": np.ascontiguousarray(C[:, bsl, :]),
            "Q": np.ascontiguousarray(Q[:, bsl, :]),
            "w4C": w4C,
            "w4Q": w4Q,
            "w4mlu": w4mlu,
        })
    res = run_bass_kernel_spmd(nc, in_maps, core_ids=list(range(NCORES)),
                               trace=trace)
    _CACHE["last_result"] = res
    outs = [res.results[i]["out"] for i in range(NCORES)]
    return np.concatenate(outs, axis=0)


# revision 22
# speedup vs baseline: 1.1556x; 1.0089x over previous
"""CQAttention (trilinear context-query attention) Bass kernel for TRN2.

Full-input contract: kernel(**inputs) takes the unsharded tensors
  C (1024, 64, 256), Q (512, 64, 256), w4C (256,1), w4Q (256,1),
  w4mlu (1,1,256), bias (1,)
and returns out (64, 1024, 1024) fp32, matching the reference

  C,Q -> batch-major; S = C@w4C + (Q@w4Q)^T + (C*w4mlu)@Q^T + bias
  S1 = softmax_q(S); S2 = softmax_c(S)
  A = S1@Q ; B = (S1@S2^T)@C
  out = concat([C, A, C*A, C*B], -1) transposed to (B, 4D, Lc)

Sharding: data-parallel over batch, 8 batch items per NeuronCore.

Algebra used on-chip (per batch item):
  * bias cancels in both softmaxes (constant shift) -> dropped.
  * e0 = exp(C@w4C), e1 = exp(Q@w4Q), X = exp((C*w4mlu)@Q^T) so that
    exp(S) = e0[c] * X[c,q] * e1[q].
  * S1 = diag(1/u) X diag(e1),  u  = X @ e1            (e0 cancels)
  * S2 = diag(e0) X diag(1/v),  v  = X^T @ e0          (e1 cancels)
  * Xe := X^T with e1 folded in during the transpose evacuation, so
    A^T   = Q^T(as lhsT) @ Xe * (1/u)-broadcast
    P2    = X^T @ [Ce | e0]; H2 = P2[:, :D] / P2[:, D]  (v folded)
    B^T   = H2(as lhsT) @ Xe * (1/u)-broadcast
  Everything is computed transposed ([feature, context] layout) so output
  DMA rows are contiguous in DRAM.

Cost-model/scheduling notes (TimelineSim):
  * matmul cost = out_free_size * pe_cycle * cyc_per_row; cyc_per_row is
    keyed on ins[0] = the MOVING (rhs) operand. fp32r >= 256 wide: 1.0.
  * transpose cost keys on the identity (rhs) dtype: f32r identity ->
    1.5 cyc/row (vs 2.0 for f32); transposes execute as exact
    permutations. (A bf16 identity would be 1.0 but neuronxcc rejects
    mixed 32/16-bit matmul inputs, NCC_IBIR034.)
  * X^T is a PE-transpose of X (32 x 53ns) instead of a second scores
    matmul (16 x 213ns).
  * sub0/sub1 matvecs are N=1 matmuls (~2ns each) instead of
    gpsimd-mult + DVE tensor_reduce.
  * software pipelining: batch b+1's transpose head (CT/QT/matvecs/Ce)
    is emitted between batch b's P1T/P3T psum groups so the PE never
    waits on transpose-evacuation engines; E0T transpose groups are
    interleaved with P2 matmul chunks for the same reason.
"""

import numpy as np

LC, LQ, B, D = 1024, 512, 64, 256
NCORES = 8
BPC = B // NCORES  # batch items per core
P = 128
MC = LC // P  # 8 context chunks
TQ = LQ // P  # 4 query chunks
KD = D // P   # 2 feature chunks

MM_RELAXED = True

_CACHE = {}


def _ensure_path():
    import sys
    for p in ("/opt/trn_rl_repo",):
        if p not in sys.path:
            sys.path.insert(0, p)


def _build_nc(mm_relaxed=MM_RELAXED):
    _ensure_path()
    import concourse.bass as bass
    import concourse.bacc as bacc
    import concourse.mybir as mybir
    from concourse import tile, masks

    f32 = mybir.dt.float32
    bf16 = mybir.dt.bfloat16
    mmdt = mybir.dt.float32r if mm_relaxed else f32
    Exp = mybir.ActivationFunctionType.Exp
    Copy = mybir.ActivationFunctionType.Copy
    mult = mybir.AluOpType.mult
    add = mybir.AluOpType.add

    def r(ap):
        return ap.bitcast(mmdt)

    nc = bacc.Bacc()
    C_d = nc.dram_tensor("C", [LC, BPC, D], f32, kind="ExternalInput")
    Q_d = nc.dram_tensor("Q", [LQ, BPC, D], f32, kind="ExternalInput")
    w4C_d = nc.dram_tensor("w4C", [D, 1], f32, kind="ExternalInput")
    w4Q_d = nc.dram_tensor("w4Q", [D, 1], f32, kind="ExternalInput")
    w4mlu_d = nc.dram_tensor("w4mlu", [1, 1, D], f32, kind="ExternalInput")
    out_d = nc.dram_tensor("out", [BPC, 4 * D, LC], f32, kind="ExternalOutput")

    with tile.TileContext(nc) as tc:
        import contextlib

        with contextlib.ExitStack() as ctx:
            ep = ctx.enter_context

            consts = ep(tc.tile_pool(name="consts", bufs=1))
            cn_pool = ep(tc.tile_pool(name="cn", bufs=2))
            cnr_pool = ep(tc.tile_pool(name="cnr", bufs=1))
            qn_pool = ep(tc.tile_pool(name="qn", bufs=2))
            ct_pool = ep(tc.tile_pool(name="ct", bufs=2))
            qt_pool = ep(tc.tile_pool(name="qt", bufs=2))
            qmt_pool = ep(tc.tile_pool(name="qmt", bufs=2))
            qe_pool = ep(tc.tile_pool(name="qe", bufs=2))
            e1b_pool = ep(tc.tile_pool(name="e1b", bufs=2))
            ce_pool = ep(tc.tile_pool(name="ce", bufs=2))
            e0_pool = ep(tc.tile_pool(name="e0p", bufs=1))
            e0t_pool = ep(tc.tile_pool(name="e0tp", bufs=2))
            h2_pool = ep(tc.tile_pool(name="h2", bufs=2))
            rsbr_pool = ep(tc.tile_pool(name="rsbr", bufs=2))
            at_pool = ep(tc.tile_pool(name="at", bufs=2))
            bt_pool = ep(tc.tile_pool(name="bt", bufs=2))
            # O2 reuses ce_pool slots (Ce dead after P2); O3 reuses e0t slots
            o2_pool = ce_pool
            o3_pool = e0t_pool
            small_pool = ep(tc.tile_pool(name="small", bufs=2))

            psA = ep(tc.tile_pool(name="psA", bufs=4, space="PSUM"))
            psB = ep(tc.tile_pool(name="psB", bufs=2, space="PSUM"))
            psRow = ep(tc.tile_pool(name="psRow", bufs=2, space="PSUM"))

            # ---- per-core constants ----
            ident = consts.tile([P, P], f32)
            masks.make_identity(nc, ident[:])
            identr = consts.tile([P, P], f32)
            nc.scalar.copy(r(identr[:]), ident[:])
            # matvec weight chunks are duplicated into column pairs so the
            # N=1 matvec matmuls get 8-byte-aligned 2-wide PSUM outputs;
            # their DMAs + broadcast copies are deferred to _consts_dmas()
            # (after the batch-0 loads) so they don't delay the first batch
            w4mlu_pp = consts.tile([P, KD], f32)
            w4Cp_s = consts.tile([P, KD], f32)
            w4Cp = consts.tile([P, 2 * KD], f32)
            w4Qp_s = consts.tile([P, KD], f32)
            w4Qp = consts.tile([P, 2 * KD], f32)

            load_state = {}
            head_state = {}

            def _consts_dmas():
                nc.sync.dma_start(
                    w4mlu_pp[:], w4mlu_d[0, 0, :].rearrange("(k p) -> p k", p=P)
                )
                nc.sync.dma_start(
                    w4Cp_s[:], w4C_d[:, 0].rearrange("(k p) -> p k", p=P)
                )
                nc.sync.dma_start(
                    w4Qp_s[:], w4Q_d[:, 0].rearrange("(k p) -> p k", p=P)
                )
                for k in range(KD):
                    nc.scalar.copy(
                        r(w4Cp[:, 2 * k:2 * k + 2]),
                        w4Cp_s[:, k:k + 1].broadcast_to([P, 2]),
                    )
                    nc.scalar.copy(
                        r(w4Qp[:, 2 * k:2 * k + 2]),
                        w4Qp_s[:, k:k + 1].broadcast_to([P, 2]),
                    )

            def _loads(b, split=False):
                # DMA loads (natural layouts) for batch b; batch 0 is split
                # per-chunk so the first transposes start sooner
                Qn = qn_pool.tile([P, TQ * D], f32, tag="qn")
                if split:
                    for t in range(TQ):
                        nc.sync.dma_start(
                            Qn[:, t * D:(t + 1) * D],
                            Q_d[t * P:(t + 1) * P, b, :],
                        )
                else:
                    nc.sync.dma_start(
                        Qn[:].rearrange("p (t d) -> p t d", t=TQ),
                        Q_d[:, b, :].rearrange("(t p) d -> p t d", p=P),
                    )
                Cn = cn_pool.tile([P, MC * D], f32, tag="cn")
                if split:
                    for mg in range(0, MC, 4):
                        nc.sync.dma_start(
                            Cn[:, mg * D:(mg + 4) * D].rearrange(
                                "p (m d) -> p m d", m=4),
                            C_d[mg * P:(mg + 4) * P, b, :].rearrange(
                                "(m p) d -> p m d", p=P),
                        )
                else:
                    nc.sync.dma_start(
                        Cn[:].rearrange("p (m d) -> p m d", m=MC),
                        C_d[:, b, :].rearrange("(m p) d -> p m d", p=P),
                    )
                load_state[b] = (Cn, Qn)

            def _head_steps(b, first=False):
                """Generator of head-phase emission steps for batch b:
                transposes CT/QT (bf16 ident), matvec matmuls, exps, QmT, Ce.
                Yields after each PE psum group so the caller can interleave
                these between other psum-heavy PE work."""
                Cn, Qn = load_state.pop(b)
                CT = ct_pool.tile([P, KD * LC], f32, tag="ct")
                QT = qt_pool.tile([P, KD * LQ], f32, tag="qt")
                # f32r-rounded copy of Cn so the CT transposes run in f32r
                # mode (1.5 cyc/row instead of 2.0); batch 0 has nothing to
                # overlap the copy with, so it keeps the f32 path
                if not first:
                    Cnr = cnr_pool.tile([P, MC * D], f32, tag="cnr")
                    nc.scalar.copy(
                        r(Cnr[:, 0:MC * D // 2]), Cn[:, 0:MC * D // 2])
                    nc.vector.tensor_copy(
                        r(Cnr[:, MC * D // 2:]), Cn[:, MC * D // 2:]
                    )
                sub0ps = psRow.tile([P, 2 * MC], f32, tag="psRow")
                sub1ps = psRow.tile([P, 2 * TQ], f32, tag="psRow")

                # QT groups first (QmT unblocks E0 of next batch)
                for k in range(KD):
                    pst = psA.tile([P, 4 * P], f32, tag="psA")
                    for t in range(TQ):
                        nc.tensor.transpose(
                            pst[:, t * P:(t + 1) * P],
                            Qn[:, t * D + k * P: t * D + (k + 1) * P],
                            ident[:],
                        )
                    nc.scalar.copy(r(QT[:, k * LQ: k * LQ + 4 * P]), pst[:])
                    yield
                for t in range(TQ):
                    for k in range(KD):
                        nc.tensor.matmul(
                            sub1ps[:, 2 * t: 2 * t + 2],
                            r(QT[:, k * LQ + t * P: k * LQ + (t + 1) * P]),
                            r(w4Qp[:, 2 * k: 2 * k + 2]),
                            start=(k == 0),
                            stop=(k == KD - 1),
                        )
                QmT = qmt_pool.tile([P, KD * LQ], f32, tag="qmt")
                for k in range(KD):
                    nc.vector.tensor_scalar_mul(
                        r(QmT[:, k * LQ:(k + 1) * LQ]),
                        QT[:, k * LQ:(k + 1) * LQ],
                        w4mlu_pp[:, k:k + 1],
                    )
                yield

                for mg in range(0, MC, 4):
                    for k in range(KD):
                        pst = psA.tile([P, 4 * P], f32, tag="psA")
                        for j in range(4):
                            m = mg + j
                            if first:
                                nc.tensor.transpose(
                                    pst[:, j * P:(j + 1) * P],
                                    Cn[:, m * D + k * P: m * D + (k + 1) * P],
                                    ident[:],
                                )
                            else:
                                nc.tensor.transpose(
                                    r(pst[:, j * P:(j + 1) * P]),
                                    r(Cnr[:, m * D + k * P:
                                          m * D + (k + 1) * P]),
                                    r(identr[:]),
                                )
                        nc.scalar.copy(
                            r(CT[:, k * LC + mg * P: k * LC + (mg + 4) * P]),
                            pst[:],
                        )
                        yield
                    for m in range(mg, mg + 4):
                        for k in range(KD):
                            nc.tensor.matmul(
                                sub0ps[:, 2 * m: 2 * m + 2],
                                r(CT[:, k * LC + m * P: k * LC + (m + 1) * P]),
                                r(w4Cp[:, 2 * k: 2 * k + 2]),
                                start=(k == 0),
                                stop=(k == KD - 1),
                            )
                # early output of C^T rows (single merged DMA)
                nc.sync.dma_start(
                    out_d[b, 0:KD * P, :].rearrange("(k p) c -> p k c", p=P),
                    CT[:].rearrange("p (k c) -> p k c", k=KD),
                )
                yield

                e1 = small_pool.tile([P, TQ], f32, tag="e1")
                nc.scalar.activation(r(e1[:]), sub1ps[:, 0:2 * TQ:2], Exp)
                e0 = small_pool.tile([P, MC], f32, tag="e0")
                nc.scalar.activation(r(e0[:]), sub0ps[:, 0:2 * MC:2], Exp)
                yield

                Qe = qe_pool.tile([P, TQ * D], f32, tag="qe")
                for t in range(TQ):
                    nc.vector.tensor_scalar_mul(
                        r(Qe[:, t * D:(t + 1) * D]), Qn[:, t * D:(t + 1) * D],
                        e1[:, t:t + 1],
                    )
                # e1 replicated across free dim: lhsT for the direct-rsbr
                # matmuls (rs broadcast across partitions in one step)
                E1B = e1b_pool.tile([P, TQ * P], f32, tag="e1b")
                for t in range(TQ):
                    nc.vector.tensor_copy(
                        r(E1B[:, t * P:(t + 1) * P]),
                        e1[:, t:t + 1].broadcast_to([P, P]),
                    )
                yield

                # Ce = C * e0 with e0 appended (cols D..D+1)
                DA = D + 2
                Ce = ce_pool.tile([P, MC * DA], f32, tag="ceA")
                for m in range(MC):
                    nc.vector.tensor_scalar_mul(
                        r(Ce[:, m * DA:m * DA + D]), Cn[:, m * D:(m + 1) * D],
                        e0[:, m:m + 1],
                    )
                    nc.vector.tensor_copy(
                        r(Ce[:, m * DA + D:m * DA + DA]),
                        e0[:, m:m + 1].broadcast_to([P, 2]),
                    )
                head_state[b] = (CT, QT, QmT, Ce, Qe, E1B, e0, e1)
                yield

            def _drain(gen):
                for _ in gen:
                    pass

            def _main(b, tail_gen):
                """Main phase for batch b; emits steps from tail_gen (the
                head of batch b+1) between its own psum groups."""
                CT, QT, QmT, Ce, Qe, E1B, e0, e1 = head_state.pop(b)
                DA = D + 2

                def _tail_step():
                    if tail_gen is not None:
                        next(tail_gen, None)

                # ---- X = exp((C*w)@Q^T) [c,(m,q)] ----
                E0 = e0_pool.tile([P, MC * LQ], f32, tag="e0")
                for m in range(MC):
                    ps = psA.tile([P, LQ], f32, tag="psA")
                    for k in range(KD):
                        nc.tensor.matmul(
                            ps[:],
                            r(CT[:, k * LC + m * P: k * LC + (m + 1) * P]),
                            r(QmT[:, k * LQ:(k + 1) * LQ]),
                            start=(k == 0),
                            stop=(k == KD - 1),
                        )
                    nc.scalar.activation(r(E0[:, m * LQ:(m + 1) * LQ]), ps[:], Exp)

                # ---- Xe = X^T * e1 via PE transposes, interleaved with
                # ---- P2 = X^T @ [Ce|e0] ; H2 = P2/(e1*v)  [q,(t,d)] ----
                E0T = e0t_pool.tile([P, TQ * LC], f32, tag="e0tA")
                H2 = h2_pool.tile([P, TQ * D], f32, tag="h2")
                rec_cse = small_pool.tile([P, TQ], f32, tag="rec")
                dq = small_pool.tile([P, TQ], f32, tag="dq")

                def _e0t_group(t, mg):
                    pst = psA.tile([P, 4 * P], f32, tag="psA")
                    for j in range(4):
                        m = mg + j
                        nc.tensor.transpose(
                            r(pst[:, j * P:(j + 1) * P]),
                            r(E0[:, m * LQ + t * P: m * LQ + (t + 1) * P]),
                            r(identr[:]),
                        )
                    osl = slice(t * LC + mg * P, t * LC + (mg + 4) * P)
                    if t % 2 == 0:
                        nc.scalar.copy(r(E0T[:, osl]), pst[:])
                    else:
                        nc.vector.tensor_copy(r(E0T[:, osl]), pst[:])

                def _p2_group(qm):
                    ps = psB.tile([P, 512], f32, tag="psB")
                    for m in range(MC):
                        nc.tensor.matmul(
                            ps[:, 0:DA],
                            r(E0[:, m * LQ + qm * P: m * LQ + (qm + 1) * P]),
                            r(Ce[:, m * DA:(m + 1) * DA]),
                            start=(m == 0),
                            stop=(m == MC - 1),
                        )
                    nc.vector.reciprocal(rec_cse[:, qm:qm + 1], ps[:, D:D + 1])
                    nc.vector.tensor_tensor(
                        dq[:, qm:qm + 1], rec_cse[:, qm:qm + 1], e1[:, qm:qm + 1],
                        mult,
                    )
                    nc.scalar.activation(
                        r(H2[:, qm * D:(qm + 1) * D]), ps[:, 0:D], Copy,
                        scale=dq[:, qm:qm + 1],
                    )

                for t in range(TQ):
                    _e0t_group(t, 0)
                    _e0t_group(t, 4)
                    _p2_group(t)

                # ---- rsbr = 1/(rs broadcast): E1B-lhsT puts rs on every
                # ---- partition directly ----
                rsbr = rsbr_pool.tile([P, LC], f32, tag="rsbr")
                for n in range(2):
                    ps = psB.tile([P, 512], f32, tag="psB")
                    for t in range(TQ):
                        nc.tensor.matmul(
                            ps[:],
                            r(E1B[:, t * P:(t + 1) * P]),
                            r(E0T[:, t * LC + n * 512: t * LC + (n + 1) * 512]),
                            start=(t == 0),
                            stop=(t == TQ - 1),
                        )
                    nc.vector.reciprocal(rsbr[:, n * 512:(n + 1) * 512], ps[:])

                # ---- A^T = Q(lhsT) @ Xe ; O2 = CT*AT ----
                AT = at_pool.tile([P, KD * LC], f32, tag="at")
                O2 = o2_pool.tile([P, KD * LC], f32, tag="ceA")
                for m2 in range(KD):
                    for n in range(2):
                        ps = psB.tile([P, 512], f32, tag="psB")
                        for t in range(TQ):
                            nc.tensor.matmul(
                                ps[:],
                                r(Qe[:, t * D + m2 * P: t * D + (m2 + 1) * P]),
                                r(E0T[:, t * LC + n * 512: t * LC + (n + 1) * 512]),
                                start=(t == 0),
                                stop=(t == TQ - 1),
                            )
                        sl = slice(m2 * LC + n * 512, m2 * LC + (n + 1) * 512)
                        nsl = slice(n * 512, (n + 1) * 512)
                        nc.vector.tensor_tensor(AT[:, sl], ps[:], rsbr[:, nsl], mult)
                        nc.gpsimd.tensor_tensor(O2[:, sl], CT[:, sl], AT[:, sl], mult)
                        nc.sync.dma_start(
                            out_d[b, 2 * P + m2 * P: 2 * P + (m2 + 1) * P, nsl],
                            AT[:, sl],
                        )
                        nc.sync.dma_start(
                            out_d[b, 4 * P + m2 * P: 4 * P + (m2 + 1) * P, nsl],
                            O2[:, sl],
                        )
                        _tail_step()
                        _tail_step()

                # ---- B^T = H2(lhsT) @ Xe ; O3 = CT*BT ----
                BT = bt_pool.tile([P, KD * LC], f32, tag="bt")
                O3 = o3_pool.tile([P, KD * LC], f32, tag="e0tA")
                for m2 in range(KD):
                    for n in range(2):
                        ps = psB.tile([P, 512], f32, tag="psB")
                        for t in range(TQ):
                            nc.tensor.matmul(
                                ps[:],
                                r(H2[:, t * D + m2 * P: t * D + (m2 + 1) * P]),
                                r(E0T[:, t * LC + n * 512: t * LC + (n + 1) * 512]),
                                start=(t == 0),
                                stop=(t == TQ - 1),
                            )
                        sl = slice(m2 * LC + n * 512, m2 * LC + (n + 1) * 512)
                        nsl = slice(n * 512, (n + 1) * 512)
                        last = tail_gen is None and m2 == KD - 1 and n == 1
                        if not last:
                            nc.vector.tensor_tensor(
                                BT[:, sl], ps[:], rsbr[:, nsl], mult)
                            nc.gpsimd.tensor_tensor(
                                O3[:, sl], CT[:, sl], BT[:, sl], mult)
                            nc.sync.dma_start(
                                out_d[b, 6 * P + m2 * P: 6 * P + (m2 + 1) * P,
                                      nsl],
                                O3[:, sl],
                            )
                        else:
                            # final chunk of the final batch: quarter-split
                            # chain on DVE to shorten the drain tail
                            for q4 in range(2):
                                qsl = slice(sl.start + q4 * 256,
                                            sl.start + (q4 + 1) * 256)
                                qnsl = slice(nsl.start + q4 * 256,
                                             nsl.start + (q4 + 1) * 256)
                                nc.vector.tensor_tensor(
                                    BT[:, qsl], ps[:, q4 * 256:(q4 + 1) * 256],
                                    rsbr[:, qnsl], mult)
                                nc.vector.tensor_tensor(
                                    O3[:, qsl], CT[:, qsl], BT[:, qsl], mult)
                                nc.sync.dma_start(
                                    out_d[b, 6 * P + m2 * P:
                                          6 * P + (m2 + 1) * P, qnsl],
                                    O3[:, qsl],
                                )
                        _tail_step()
                        _tail_step()

                # finish any remaining head steps of b+1
                if tail_gen is not None:
                    _drain(tail_gen)

            _loads(0, split=True)
            _consts_dmas()
            _drain(_head_steps(0, first=True))
            for b in range(BPC):
                if b + 1 < BPC:
                    _loads(b + 1)
                    _main(b, _head_steps(b + 1))
                else:
                    _main(b, None)

    nc.compile()
    return nc


def _get_nc(mm_relaxed=MM_RELAXED):
    key = ("nc", mm_relaxed)
    if key not in _CACHE:
        _CACHE[key] = _build_nc(mm_relaxed)
    return _CACHE[key]


def kernel(C, Q, w4C, w4Q, w4mlu, bias=None, trace=False, **_ignored):
    _ensure_path()
    from concourse.bass_utils import run_bass_kernel_spmd

    C = np.ascontiguousarray(np.asarray(C, dtype=np.float32))
    Q = np.ascontiguousarray(np.asarray(Q, dtype=np.float32))
    w4C = np.ascontiguousarray(np.asarray(w4C, dtype=np.float32))
    w4Q = np.ascontiguousarray(np.asarray(w4Q, dtype=np.float32))
    w4mlu = np.ascontiguousarray(np.asarray(w4mlu, dtype=np.float32))

    nc = _get_nc()
    in_maps = []
    for i in range(NCORES):
        bsl = slice(i * BPC, (i + 1) * BPC)
        in_maps.append({
            "C": np.ascontiguousarray(C[:, bsl, :]),
            "Q": np.ascontiguousarray(Q[:, bsl, :]),
            "w4C": w4C,
            "w4Q": w4Q,
            "w4mlu": w4mlu,
        })
    res = run_bass_kernel_spmd(nc, in_maps, core_ids=list(range(NCORES)),
                               trace=trace)
    _CACHE["last_result"] = res
    outs = [res.results[i]["out"] for i in range(NCORES)]
    return np.concatenate(outs, axis=0)
